# revision 1
# baseline (speedup 1.0000x reference)
"""Trainium2 Bass kernel for GroupedQueryAttention (inverted sliding-window mask + sink).

Full inputs in, full output out. Internally head-sharded across 8 NeuronCores:
core c handles q heads {2c, 2c+1} and kv head c//2, computes its partial
(x @ Wqkv_slice -> RoPE -> scores -> masked softmax w/ sink -> AV -> @ Wo_slice),
host sums the 8 partial outputs (the all-reduce).
"""

import os
import sys
from contextlib import ExitStack

sys.path.insert(0, "/opt/trn_rl_repo")

# jax must see the axon/neuron platform; a stray JAX_PLATFORMS=cpu would hide it.
if os.environ.get("JAX_PLATFORMS", "") == "cpu":
    os.environ["JAX_PLATFORMS"] = ""

import numpy as np

import concourse.bass as bass
import concourse.tile as tile
from concourse import bacc, mybir

F32 = mybir.dt.float32
F32R = mybir.dt.float32r

N_CORES = 8
L = 2048
D = 2048
HD = 128
WINDOW = 1024
ROPE_BASE = 1024.0
SM_SCALE = 1.0 / float(np.sqrt(HD))
MASK_VAL = -1.0e5

QB = 512          # q block (free dim of score tiles)
NQB = L // QB     # 4
NKT = L // HD     # 16 k tiles of 128
NDK = D // HD     # 16 contraction chunks for projections
NLB = L // QB     # 4 l-blocks for projection

# additive-mask tiles are keyed by diff0 = q0 - k0 of the (k-tile, q-block) pair
MASK_DIFF0S = [0, -128, -256, -384, 640, 768, 896, 1024]
MASK_IDX = {d: i for i, d in enumerate(MASK_DIFF0S)}


def _classify(kt: int, qb: int):
    """masked band is 0 <= q-k <= WINDOW-1 (those entries get -inf)."""
    d0 = QB * qb - HD * kt
    if 128 <= d0 <= 512:
        return "skip", None      # tile entirely inside the band -> contributes 0
    if d0 <= -512 or d0 >= 1152:
        return "full", None      # tile entirely outside the band -> no mask needed
    return "partial", MASK_IDX[d0]


def _build_program():
    nc = bacc.Bacc("TRN2", target_bir_lowering=False, debug=False,
                   num_devices=N_CORES)

    xT_d = nc.dram_tensor("xT", [D, L], F32R, kind="ExternalInput").ap()
    wslc_d = nc.dram_tensor("wslc", [D, 4 * HD], F32R, kind="ExternalInput").ap()
    wo_d = nc.dram_tensor("wo", [2 * HD, D], F32R, kind="ExternalInput").ap()
    snk_d = nc.dram_tensor("snk", [1, 2], F32, kind="ExternalInput").ap()
    cosd_d = nc.dram_tensor("cosd", [128, L], F32, kind="ExternalInput").ap()
    sind_d = nc.dram_tensor("sind", [128, L], F32, kind="ExternalInput").ap()
    y_d = nc.dram_tensor("y", [L, D], F32, kind="ExternalOutput").ap()

    with tile.TileContext(nc) as tc, ExitStack() as stk:
        persist = stk.enter_context(tc.tile_pool(name="persist", bufs=1))

        # ---- persistent SBUF tensors ----
        wslc_sb = persist.tile([128, NDK, 4 * HD], F32R, tag="wslc")
        wo_sb = persist.tile([128, 2, D], F32R, tag="wo")
        qT = [persist.tile([128, L], F32R, tag=f"qT{h}", name=f"qT{h}") for h in range(2)]
        kT = persist.tile([128, L], F32R, tag="kT")
        vT = persist.tile([128, L], F32, tag="vT")
        v_sb = persist.tile([128, NKT, HD], F32R, tag="v")
        oT = [persist.tile([128, L], F32R, tag=f"oT{h}", name=f"oT{h}") for h in range(2)]
        cosd_sb = persist.tile([128, L], F32, tag="cosd")
        sind_sb = persist.tile([128, L], F32, tag="sind")
        masks = persist.tile([128, len(MASK_DIFF0S), QB], F32, tag="masks")
        ident = persist.tile([128, 128], F32, tag="ident")
        ones_f32 = persist.tile([128, 1], F32, tag="onesf")
        ones_sb = persist.tile([128, 1], F32R, tag="ones")
        snk_sb = persist.tile([1, 2], F32, tag="snk")
        exps_sb = persist.tile([1, 2], F32, tag="exps")

        # ---- constant / setup ops ----
        for k in range(NDK):
            nc.gpsimd.dma_start(wslc_sb[:, k, :], wslc_d[k * 128:(k + 1) * 128, :])
        nc.gpsimd.dma_start(cosd_sb[:], cosd_d[:])
        nc.gpsimd.dma_start(sind_sb[:], sind_d[:])
        nc.gpsimd.dma_start(snk_sb[:], snk_d[:])
        for h in range(2):
            nc.gpsimd.dma_start(wo_sb[:, h, :], wo_d[h * 128:(h + 1) * 128, :])

        nc.gpsimd.memset(ones_f32[:], 1.0)
        nc.scalar.copy(ones_sb[:], ones_f32[:])
        # identity for PE transposes
        nc.gpsimd.memset(ident[:], 0.0)
        nc.gpsimd.affine_select(
            out=ident[:], in_=ident[:], compare_op=mybir.AluOpType.not_equal,
            fill=1.0, base=0, channel_multiplier=1, pattern=[[-1, 128]])
        # additive mask tiles: -1e5 where 0 <= (q-k) <= WINDOW-1, else 0
        for i, d0 in enumerate(MASK_DIFF0S):
            m = masks[:, i, :]
            nc.gpsimd.memset(m, 0.0)
            # keep 0 where k-q-1 >= 0 (i.e. q-k < 0), else fill MASK_VAL
            nc.gpsimd.affine_select(
                out=m, in_=m, compare_op=mybir.AluOpType.is_ge,
                fill=MASK_VAL, base=-d0 - 1, channel_multiplier=1,
                pattern=[[-1, QB]])
            # keep where (WINDOW-1)-(q-k) >= 0 (i.e. q-k < WINDOW), else fill 0
            nc.gpsimd.affine_select(
                out=m, in_=m, compare_op=mybir.AluOpType.is_ge,
                fill=0.0, base=WINDOW - 1 - d0, channel_multiplier=1,
                pattern=[[-1, QB]])
        # exp of the two sink logits
        nc.scalar.activation(exps_sb[:], snk_sb[:], mybir.ActivationFunctionType.Exp)

        # ================= Phase A: QKV projection (transposed) =================
        # pT[c*128+r, l] = sum_d wslc[d, c*128+r] * x[l, d];  cols c = q0,q1,k,v
        col_dst = [qT[0], qT[1], kT, vT]
        with tc.tile_pool(name="psA", bufs=8, space="PSUM") as psA, \
             tc.tile_pool(name="xt", bufs=4) as xt_pool:
            for lb in range(NLB):
                psums = [psA.tile([128, QB], F32, tag="proj", name=f"psproj{c}") for c in range(4)]
                for k in range(NDK):
                    xt = xt_pool.tile([128, QB], F32R, tag="xt")
                    nc.sync.dma_start(
                        xt[:], xT_d[k * 128:(k + 1) * 128, lb * QB:(lb + 1) * QB])
                    for c in range(4):
                        nc.tensor.matmul(
                            psums[c][:],
                            wslc_sb[:, k, c * 128:(c + 1) * 128],
                            xt[:],
                            start=(k == 0), stop=(k == NDK - 1))
                for c in range(4):
                    nc.scalar.copy(col_dst[c][:, lb * QB:(lb + 1) * QB], psums[c][:])

            # ---- RoPE on qT[0], qT[1], kT (in place, transposed layout) ----
            with tc.tile_pool(name="rope", bufs=2) as rope_pool:
                for t in (qT[0], qT[1], kT):
                    partner = rope_pool.tile([128, L], F32, tag="partner")
                    nc.gpsimd.memset(partner[32:64, :], 0.0)
                    nc.gpsimd.memset(partner[96:128, :], 0.0)
                    nc.gpsimd.dma_start(partner[0:32, :], t[64:96, :].bitcast(F32))
                    nc.gpsimd.dma_start(partner[64:96, :], t[0:32, :].bitcast(F32))
                    tmp = rope_pool.tile([128, L], F32, tag="ropetmp")
                    nc.vector.tensor_mul(tmp[:], t[:], cosd_sb[:])
                    nc.vector.tensor_mul(partner[:], partner[:], sind_sb[:])
                    nc.vector.tensor_add(t[:], tmp[:], partner[:])

            # ---- v: transpose to natural (k, d) tiles ----
            for t in range(NKT):
                pt = psA.tile([128, 128], F32, tag="proj")
                nc.tensor.transpose(pt[:], vT[:, t * 128:(t + 1) * 128], ident[:])
                nc.scalar.copy(v_sb[:, t, :], pt[:])

        # ============ Phase B+C: attention + output projection ============
        with tc.tile_pool(name="psS", bufs=3, space="PSUM") as psS, \
             tc.tile_pool(name="psO", bufs=2, space="PSUM") as psO, \
             tc.tile_pool(name="psD", bufs=1, space="PSUM") as psD, \
             tc.tile_pool(name="psY", bufs=2, space="PSUM") as psY, \
             tc.tile_pool(name="sbB", bufs=6) as sbB, \
             tc.tile_pool(name="sbY", bufs=4) as sbY:
            for qb in range(NQB):
                qs = slice(qb * QB, (qb + 1) * QB)
                for h in range(2):
                    acts = [(kt, _classify(kt, qb)) for kt in range(NKT)]
                    acts = [(kt, c, mi) for kt, (c, mi) in acts if c != "skip"]
                    n_act = len(acts)
                    psum_o = psO.tile([128, QB], F32, tag="o")
                    psum_den = psD.tile([1, QB], F32, tag="den")
                    for i, (kt, cls, mi) in enumerate(acts):
                        psum_s = psS.tile([128, QB], F32, tag="s")
                        nc.tensor.matmul(
                            psum_s[:],
                            kT[:, kt * 128:(kt + 1) * 128],
                            qT[h][:, qs],
                            start=True, stop=True)
                        e_sb = sbB.tile([128, QB], F32R, tag="e")
                        if cls == "partial":
                            s_sb = sbB.tile([128, QB], F32, tag="smask")
                            nc.vector.tensor_add(s_sb[:], psum_s[:], masks[:, mi, :])
                            src = s_sb
                        else:
                            src = psum_s
                        nc.scalar.activation(
                            e_sb[:], src[:], mybir.ActivationFunctionType.Exp,
                            scale=SM_SCALE)
                        nc.tensor.matmul(
                            psum_den[:], ones_sb[:],
                            e_sb[:],
                            start=(i == 0), stop=(i == n_act - 1))
                        nc.tensor.matmul(
                            psum_o[:], v_sb[:, kt, :],
                            e_sb[:],
                            start=(i == 0), stop=(i == n_act - 1))
                    den_sb = sbB.tile([1, QB], F32, tag="densb")
                    nc.scalar.activation(
                        den_sb[:], psum_den[:],
                        mybir.ActivationFunctionType.Identity,
                        bias=exps_sb[0:1, h:h + 1])
                    r_sb = sbB.tile([1, QB], F32, tag="rsb")
                    nc.vector.reciprocal(r_sb[:], den_sb[:])
                    rb = sbB.tile([128, QB], F32, tag="rb")
                    nc.gpsimd.partition_broadcast(rb[:], r_sb[:])
                    nc.vector.tensor_mul(oT[h][:, qs], psum_o[:], rb[:])

                # ---- Wo for this q block ----
                for j in range(QB // 128):
                    qt = qb * (QB // 128) + j
                    qts = slice(qt * 128, (qt + 1) * 128)
                    for nb in range(D // QB):
                        ns = slice(nb * QB, (nb + 1) * QB)
                        psum_y = psY.tile([128, QB], F32, tag="y")
                        for h in range(2):
                            nc.tensor.matmul(
                                psum_y[:],
                                oT[h][:, qts],
                                wo_sb[:, h, ns],
                                start=(h == 0), stop=(h == 1))
                        y_sb = sbY.tile([128, QB], F32, tag="ysb")
                        if (qt + nb) % 2 == 0:
                            nc.scalar.copy(y_sb[:], psum_y[:])
                        else:
                            nc.vector.tensor_copy(y_sb[:], psum_y[:])
                        nc.sync.dma_start(y_d[qts, ns], y_sb[:])

    nc.compile()
    return nc


def _rope_tables():
    freqs = (1.0 / ROPE_BASE) ** np.linspace(0.0, 1.0, num=HD // 4,
                                             dtype=np.float32)
    theta = freqs[:, None].astype(np.float32) * np.arange(L, dtype=np.float32)[None, :]
    cos32 = np.cos(theta).astype(np.float32)
    sin32 = np.sin(theta).astype(np.float32)
    cosd = np.ones((128, L), dtype=np.float32)
    sind = np.zeros((128, L), dtype=np.float32)
    cosd[0:32] = cos32
    cosd[64:96] = cos32
    sind[0:32] = sin32
    sind[64:96] = -sin32
    return cosd, sind


def _make_in_maps(x, Wqkv, Wo, s):
    x = np.asarray(x, dtype=np.float32)
    Wqkv = np.asarray(Wqkv, dtype=np.float32)
    Wo = np.asarray(Wo, dtype=np.float32)
    s = np.asarray(s, dtype=np.float32)
    xT = np.ascontiguousarray(x.reshape(L, D).T)
    cosd, sind = _rope_tables()
    in_maps = []
    for c in range(N_CORES):
        g = c // 2
        wslc = np.concatenate([
            Wqkv[:, (2 * c) * HD:(2 * c + 2) * HD],
            Wqkv[:, 16 * HD + g * HD:16 * HD + (g + 1) * HD],
            Wqkv[:, 20 * HD + g * HD:20 * HD + (g + 1) * HD],
        ], axis=1)
        in_maps.append({
            "xT": xT,
            "wslc": np.ascontiguousarray(wslc),
            "wo": np.ascontiguousarray(Wo[(2 * c) * HD:(2 * c + 2) * HD, :]),
            "snk": np.ascontiguousarray(s[:, 2 * c:2 * c + 2]),
            "cosd": cosd,
            "sind": sind,
        })
    return in_maps


_CACHE = {}


def _get_exec():
    """Build the program once and return a cached jitted 8-core executor."""
    if "exec" in _CACHE:
        return _CACHE["exec"]

    import jax
    from jax.sharding import Mesh, PartitionSpec
    from jax.experimental.shard_map import shard_map
    from concourse.bass2jax import (_bass_exec_p, install_neuronx_cc_hook,
                                    partition_id_tensor)

    nc = _build_program()
    install_neuronx_cc_hook()

    partition_name = (nc.partition_id_tensor.name
                      if nc.partition_id_tensor else None)
    in_names, out_names, out_avals = [], [], []
    for alloc in nc.m.functions[0].allocations:
        if not isinstance(alloc, mybir.MemoryLocationSet):
            continue
        name = alloc.memorylocations[0].name
        if alloc.kind == "ExternalInput":
            if name != partition_name:
                in_names.append(name)
        elif alloc.kind == "ExternalOutput":
            out_names.append(name)
            out_avals.append(jax.core.ShapedArray(
                tuple(alloc.tensor_shape), mybir.dt.np(alloc.dtype)))
    n_params = len(in_names)
    all_names = in_names + out_names
    if partition_name is not None:
        all_names = all_names + [partition_name]

    def _body(*args):
        operands = list(args)
        if partition_name is not None:
            operands.append(partition_id_tensor())
        outs = _bass_exec_p.bind(
            *operands,
            out_avals=tuple(out_avals),
            in_names=tuple(all_names),
            out_names=tuple(out_names),
            lowering_input_output_aliases=(),
            sim_require_finite=True,
            sim_require_nnan=True,
            nc=nc,
        )
        return tuple(outs)

    devices = jax.devices()[:N_CORES]
    mesh = Mesh(np.asarray(devices), ("core",))
    n_outs = len(out_names)
    sharded = jax.jit(
        shard_map(_body, mesh=mesh,
                  in_specs=(PartitionSpec("core"),) * (n_params + n_outs),
                  out_specs=(PartitionSpec("core"),) * n_outs,
                  check_rep=False),
        keep_unused=True)

    state = {
        "sharded": sharded, "in_names": in_names, "out_names": out_names,
        "out_avals": out_avals, "mesh": mesh, "n_params": n_params,
    }
    _CACHE["exec"] = state
    return state


def _run_cores(in_maps):
    ex = _get_exec()
    concat_in = [
        np.concatenate([np.asarray(m[name]) for m in in_maps], axis=0)
        for name in ex["in_names"]
    ]
    concat_zeros = [
        np.zeros((N_CORES * a.shape[0],) + tuple(a.shape[1:]), a.dtype)
        for a in ex["out_avals"]
    ]
    outs = ex["sharded"](*concat_in, *concat_zeros)
    name_to_i = {n: i for i, n in enumerate(ex["out_names"])}
    yi = name_to_i["y"]
    y_all = np.asarray(outs[yi]).reshape(N_CORES, L, D)
    return y_all


def kernel(x, Wqkv, Wo, s):
    in_maps = _make_in_maps(x, Wqkv, Wo, s)
    y_all = _run_cores(in_maps)
    out = y_all.sum(axis=0, dtype=np.float32)
    return out.reshape(1, L, D).astype(np.float32)



# revision 30
# speedup vs baseline: 1.2472x; 1.2472x over previous
"""Trainium2 Bass kernel for GroupedQueryAttention (inverted sliding-window mask + sink).

Full inputs in, full output out. Internally head-sharded across 8 NeuronCores:
core c handles q heads {2c, 2c+1} and kv head c//2, computes its partial
(x @ Wqkv_slice -> RoPE -> scores -> masked softmax w/ sink -> AV -> @ Wo_slice),
host sums the 8 partial outputs (the all-reduce).

v2: bf16 matmul inputs (host-cast), streamed x loads, post-exp multiplicative
masks on DVE, stream_shuffle RoPE, Wo tiles interleaved into next q-block.
"""

import os
import sys
from contextlib import ExitStack

sys.path.insert(0, "/opt/trn_rl_repo")

# jax must see the axon/neuron platform; a stray JAX_PLATFORMS=cpu would hide it.
if os.environ.get("JAX_PLATFORMS", "") == "cpu":
    os.environ["JAX_PLATFORMS"] = ""

import numpy as np

import concourse.bass as bass
import concourse.tile as tile
from concourse import bacc, mybir

F32 = mybir.dt.float32
BF16 = mybir.dt.bfloat16

N_CORES = 8
L = 2048
D = 2048
HD = 128
WINDOW = 1024
ROPE_BASE = 1024.0
SM_SCALE = 1.0 / float(np.sqrt(HD))

QB = 512          # q block (free dim of score tiles)
NQB = L // QB     # 4
NKT = L // HD     # 16 k tiles of 128
NDK = D // HD     # 16 contraction chunks for projections
NLB = L // QB     # 4 l-blocks for projection

# multiplicative-mask tiles are keyed by diff0 = q0 - k0 of the (k-tile, q-block)
MASK_DIFF0S = [0, -128, -256, -384, 640, 768, 896, 1024]
MASK_IDX = {d: i for i, d in enumerate(MASK_DIFF0S)}

# stream_shuffle permutes only within 32-partition quadrants, so q/k head
# dims are re-ordered (host side) to put each RoPE pair (d, d+64) 16 rows
# apart inside one quadrant: row 32s+i -> dim 16s+i, row 32s+16+i -> dim
# 64+16s+i (i<16). The shared permutation leaves q.k dot products unchanged.
SHUF16 = [(i + 16) % 32 for i in range(32)]
ROPE_PERM = np.array(
    [16 * s + i if i < 16 else 64 + 16 * s + (i - 16)
     for s in range(4) for i in range(32)])


def _classify(kt: int, qb: int):
    """masked band is 0 <= q-k <= WINDOW-1 (those entries get zeroed)."""
    d0 = QB * qb - HD * kt
    if 128 <= d0 <= 512:
        return "skip", None      # tile entirely inside the band -> contributes 0
    if d0 <= -512 or d0 >= 1152:
        return "full", None      # tile entirely outside the band -> no mask needed
    return "partial", MASK_IDX[d0]


def _build_program(dump=False):
    nc = bacc.Bacc("TRN2", target_bir_lowering=False, debug=False,
                   num_devices=N_CORES)
    dbg = {}
    if dump:
        for nm in ("dbg_q0", "dbg_q1", "dbg_k", "dbg_vT", "dbg_o0", "dbg_o1"):
            dbg[nm] = nc.dram_tensor(nm, [128, L], F32, kind="ExternalOutput").ap()
        dbg["dbg_v"] = nc.dram_tensor("dbg_v", [128, NKT * HD], F32,
                                      kind="ExternalOutput").ap()

    # host-packed to SBUF layout: [partition, chunk, col]
    xT_d = nc.dram_tensor("xT", [128, NDK, L], BF16, kind="ExternalInput").ap()
    wslc_d = nc.dram_tensor("wslc", [128, NDK, 4 * HD], BF16,
                            kind="ExternalInput").ap()
    wo_d = nc.dram_tensor("wo", [2 * HD, D], BF16, kind="ExternalInput").ap()
    snk_d = nc.dram_tensor("snk", [1, 2], F32, kind="ExternalInput").ap()
    cosd_d = nc.dram_tensor("cosd", [128, L], BF16, kind="ExternalInput").ap()
    sind_d = nc.dram_tensor("sind", [128, L], BF16, kind="ExternalInput").ap()
    y_d = nc.dram_tensor("y", [L, D], F32, kind="ExternalOutput").ap()

    with tile.TileContext(nc) as tc, ExitStack() as stk:
        persist = stk.enter_context(tc.tile_pool(name="persist", bufs=1))

        # ---- persistent SBUF tensors ----
        wslc_sb = persist.tile([128, NDK, 4 * HD], BF16, tag="wslc")
        wo_sb = persist.tile([128, 2, D], BF16, tag="wo")
        qT = [persist.tile([128, L], BF16, tag=f"qT{h}", name=f"qT{h}") for h in range(2)]
        kT = persist.tile([128, L], BF16, tag="kT")
        vT = persist.tile([128, L], BF16, tag="vT")
        v_sb = persist.tile([128, NKT, HD], BF16, tag="v")
        oT = [persist.tile([128, L], BF16, tag=f"oT{h}", name=f"oT{h}") for h in range(2)]
        cosd_sb = persist.tile([128, L], BF16, tag="cosd")
        sind_sb = persist.tile([128, L], BF16, tag="sind")
        masks = persist.tile([128, len(MASK_DIFF0S), QB], BF16, tag="masks")
        ident = persist.tile([128, 128], BF16, tag="ident")
        ones_bf = persist.tile([128, 1], BF16, tag="ones")
        snk_sb = persist.tile([1, 2], F32, tag="snk")
        exps_sb = persist.tile([1, 2], F32, tag="exps")

        # ---- weight loads ----
        # wslc quarters on SP (chunk order matches proj consumption); first
        # chunk alone so PE can start early.
        nc.sync.dma_start(wslc_sb[:, 0, :], wslc_d[:, 0, :])
        nc.sync.dma_start(wslc_sb[:, 1:4, :], wslc_d[:, 1:4, :])
        nc.sync.dma_start(wslc_sb[:, 4:10, :], wslc_d[:, 4:10, :])
        nc.sync.dma_start(wslc_sb[:, 10:16, :], wslc_d[:, 10:16, :])
        # cos/sin (needed ~14us in for RoPE) + wo/snk on Act queue (idle early)
        nc.scalar.dma_start(cosd_sb[:], cosd_d[:])
        nc.scalar.dma_start(sind_sb[:], sind_d[:])
        nc.scalar.dma_start(snk_sb[:], snk_d[:])
        for h in range(2):
            nc.scalar.dma_start(wo_sb[:, h, :], wo_d[h * 128:(h + 1) * 128, :])

        nc.vector.memset(ones_bf[:], 1.0)
        # exp of the two sink logits
        nc.scalar.activation(exps_sb[:], snk_sb[:], mybir.ActivationFunctionType.Exp)

        def build_masks():
            # multiplicative masks: 0 where 0 <= (q-k) <= WINDOW-1, else 1
            # (on Pool, emitted mid-phase-A: needed only at phase B)
            for i, d0 in enumerate(MASK_DIFF0S):
                m = masks[:, i, :]
                nc.gpsimd.memset(m, 1.0)
                if d0 <= 0:
                    # keep 1 where q-k < 0, i.e. kp - qf - d0 - 1 >= 0
                    nc.gpsimd.affine_select(
                        out=m, in_=m, compare_op=mybir.AluOpType.is_ge,
                        fill=0.0, base=-d0 - 1, channel_multiplier=1,
                        pattern=[[-1, QB]])
                else:
                    # keep 1 where q-k >= WINDOW, i.e. qf - kp + d0 - WINDOW >= 0
                    nc.gpsimd.affine_select(
                        out=m, in_=m, compare_op=mybir.AluOpType.is_ge,
                        fill=0.0, base=d0 - WINDOW, channel_multiplier=-1,
                        pattern=[[1, QB]])

        # ================= Phase A: QKV projection (transposed) =================
        # pT[c*128+r, l] = sum_d wslc[d, c*128+r] * x[l, d];  cols c = q0,q1,k,v
        col_dst = [qT[0], qT[1], kT, vT]

        xt_sb = persist.tile([128, NDK, L], BF16, tag="xt")

        with tc.tile_pool(name="psA", bufs=6, space="PSUM") as psA, \
             tc.tile_pool(name="psT", bufs=2, space="PSUM") as psT, \
             tc.tile_pool(name="rope", bufs=4) as rp:

            def rope_slice(t, lb):
                ls = slice(lb * QB, (lb + 1) * QB)
                partner = rp.tile([128, QB], BF16, tag="partner")
                nc.vector.stream_shuffle(partner[:], t[:, ls], SHUF16)
                tmp = rp.tile([128, QB], BF16, tag="ropetmp")
                nc.vector.tensor_mul(tmp[:], t[:, ls], cosd_sb[:, ls])
                nc.vector.tensor_mul(partner[:], partner[:], sind_sb[:, ls])
                nc.vector.tensor_add(t[:, ls], tmp[:], partner[:])

            def v_transposes(lb):
                pt = psT.tile([128, 4, HD], BF16, tag="vt")
                for j in range(4):
                    t = 4 * lb + j
                    nc.tensor.transpose(
                        pt[:, j, :], vT[:, t * 128:(t + 1) * 128], ident[:])
                nc.scalar.copy(v_sb[:, 4 * lb:4 * lb + 4, :], pt[:])

            prev_v_lb = None
            for lb in range(NLB):
                ls = slice(lb * QB, (lb + 1) * QB)
                # x loads ride Pool's software DGE: the transfer is async, so
                # the Pool engine is only held ~1us per DMA.
                for g in range(4):
                    nc.gpsimd.dma_start(
                        xt_sb[:, 4 * g:4 * g + 4, ls],
                        xT_d[:, 4 * g:4 * g + 4, ls])
                if lb == 0:
                    # identity for PE transposes (needed from lb1 on)
                    nc.gpsimd.memset(ident[:], 0.0)
                    nc.gpsimd.affine_select(
                        out=ident[:], in_=ident[:],
                        compare_op=mybir.AluOpType.not_equal,
                        fill=1.0, base=0, channel_multiplier=1,
                        pattern=[[-1, 128]])
                if lb == NLB - 1:
                    build_masks()
                psums = [psA.tile([128, QB], F32, tag="proj",
                                  name=f"psproj{c}") for c in range(4)]
                for k in range(NDK):
                    for c in range(4):
                        nc.tensor.matmul(
                            psums[c][:],
                            wslc_sb[:, k, c * 128:(c + 1) * 128],
                            xt_sb[:, k, ls],
                            start=(k == 0), stop=(k == NDK - 1))
                if prev_v_lb is not None:
                    v_transposes(prev_v_lb)
                # copies psum -> bf16 SBUF; q0,k on Act; q1,v on DVE
                nc.scalar.copy(qT[0][:, ls], psums[0][:])
                nc.vector.tensor_copy(qT[1][:, ls], psums[1][:])
                nc.scalar.copy(kT[:, ls], psums[2][:])
                nc.vector.tensor_copy(vT[:, ls], psums[3][:])
                # RoPE on this slice (DVE), overlapped with next lb's matmuls
                rope_slice(kT, lb)
                rope_slice(qT[0], lb)
                rope_slice(qT[1], lb)
                prev_v_lb = lb
            v_transposes(prev_v_lb)

        # ============ Phase B+C: attention + output projection ============
        ycnt = [0]
        ystage = {}

        def make_emit_y(psY, sbY, act_share):
            def emit_y_tile(qt, nb):
                # one [128,512] y tile: 2 Wo matmuls + copy into a per-row
                # staging buffer; the whole [128,2048] row block ships as one
                # DMA (SP holds its SEQ for the full transfer; Pool's SWDGE
                # is async, so alternate).
                qts = slice(qt * 128, (qt + 1) * 128)
                ns = slice(nb * QB, (nb + 1) * QB)
                if nb == 0:
                    ystage[qt] = sbY.tile([128, D // QB, QB], F32, tag="ysb",
                                          name=f"ystage{qt}")
                psum_y = psY.tile([128, QB], F32, tag="y")
                for h in range(2):
                    nc.tensor.matmul(
                        psum_y[:],
                        oT[h][:, qts],
                        wo_sb[:, h, ns],
                        start=(h == 0), stop=(h == 1))
                # in-group copies stay off Act (exp is PE's critical feed)
                if act_share and ycnt[0] % 2 == 0:
                    nc.scalar.copy(ystage[qt][:, nb, :], psum_y[:])
                else:
                    nc.vector.tensor_copy(ystage[qt][:, nb, :], psum_y[:])
                ycnt[0] += 1
                if nb == D // QB - 1:
                    eng = nc.sync if qt % 2 == 0 else nc.gpsimd
                    eng.dma_start(y_d[qts, :], ystage[qt][:])
                    del ystage[qt]
            return emit_y_tile

        with tc.tile_pool(name="psS", bufs=4, space="PSUM") as psS, \
             tc.tile_pool(name="psO", bufs=2, space="PSUM") as psO, \
             tc.tile_pool(name="psD", bufs=1, space="PSUM") as psD, \
             tc.tile_pool(name="psY", bufs=1, space="PSUM") as psY, \
             tc.tile_pool(name="epool", bufs=8) as epool, \
             tc.tile_pool(name="sbB", bufs=4) as sbB, \
             tc.tile_pool(name="sbY", bufs=3) as sbY:

            emit_y_tile = make_emit_y(psY, sbY, act_share=False)

            pending_y = []
            LAG = 3  # tiles between score emission and its den/AV, hiding exp
            for qb in range(NQB):
                qs = slice(qb * QB, (qb + 1) * QB)
                for h in range(2):
                    acts = [(kt, _classify(kt, qb)) for kt in range(NKT)]
                    acts = [(kt, c, mi) for kt, (c, mi) in acts if c != "skip"]
                    n_act = len(acts)
                    psum_o = psO.tile([128, QB], F32, tag="o")
                    psum_den = psD.tile([1, QB], F32, tag="den")
                    e_use = [None] * n_act
                    for i in range(n_act + LAG):
                        if i < n_act:
                            kt, cls, mi = acts[i]
                            psum_s = psS.tile([128, QB], F32, tag="s")
                            nc.tensor.matmul(
                                psum_s[:],
                                kT[:, kt * 128:(kt + 1) * 128],
                                qT[h][:, qs],
                                start=True, stop=True)
                            e_sb = epool.tile([128, QB], BF16, tag="e")
                            nc.scalar.activation(
                                e_sb[:], psum_s[:],
                                mybir.ActivationFunctionType.Exp,
                                scale=SM_SCALE)
                            if cls == "partial":
                                e_m = epool.tile([128, QB], BF16, tag="em")
                                nc.vector.tensor_mul(
                                    e_m[:], e_sb[:], masks[:, mi, :])
                                e_use[i] = e_m
                            else:
                                e_use[i] = e_sb
                        j = i - LAG
                        if 0 <= j < n_act:
                            ktj = acts[j][0]
                            nc.tensor.matmul(
                                psum_den[:], ones_bf[:],
                                e_use[j][:],
                                start=(j == 0), stop=(j == n_act - 1))
                            nc.tensor.matmul(
                                psum_o[:], v_sb[:, ktj, :],
                                e_use[j][:],
                                start=(j == 0), stop=(j == n_act - 1))
                            e_use[j] = None
                        # interleave one deferred y tile of the previous q block
                        if pending_y and i >= 2:
                            emit_y_tile(*pending_y.pop(0))
                    den_sb = sbB.tile([1, QB], F32, tag="densb")
                    nc.scalar.activation(
                        den_sb[:], psum_den[:],
                        mybir.ActivationFunctionType.Identity,
                        bias=exps_sb[0:1, h:h + 1])
                    r_sb = sbB.tile([1, QB], F32, tag="rsb")
                    nc.vector.reciprocal(r_sb[:], den_sb[:])
                    rb = sbB.tile([128, QB], F32, tag="rb")
                    nc.gpsimd.partition_broadcast(rb[:], r_sb[:])
                    nc.vector.tensor_mul(oT[h][:, qs], psum_o[:], rb[:])
                pending_y.extend(
                    (qb * (QB // 128) + j, nb)
                    for j in range(QB // 128) for nb in range(D // QB))

        # final-qb y drain: attention pools are done, so rebuild with deep
        # buffering and let the copies use both Act and DVE
        with tc.tile_pool(name="psY2", bufs=4, space="PSUM") as psY2, \
             tc.tile_pool(name="sbY2", bufs=3) as sbY2:
            emit_y_tile = make_emit_y(psY2, sbY2, act_share=True)
            while pending_y:
                emit_y_tile(*pending_y.pop(0))

        if dump:
            with tc.tile_pool(name="dbgp", bufs=2) as dbgp:
                for nm, t in (("dbg_q0", qT[0]), ("dbg_q1", qT[1]),
                              ("dbg_k", kT), ("dbg_vT", vT),
                              ("dbg_o0", oT[0]), ("dbg_o1", oT[1])):
                    f = dbgp.tile([128, L], F32, tag="dbgf", name=f"f{nm}")
                    nc.scalar.copy(f[:], t[:])
                    nc.sync.dma_start(dbg[nm], f[:])
                fv = dbgp.tile([128, NKT, HD], F32, tag="dbgf", name="fv")
                nc.scalar.copy(fv[:], v_sb[:])
                nc.sync.dma_start(dbg["dbg_v"], fv[:])

    nc.compile()
    return nc


def _rope_tables():
    """cos/sin tables in the permuted row order (see ROPE_PERM)."""
    freqs = (1.0 / ROPE_BASE) ** np.linspace(0.0, 1.0, num=HD // 4,
                                             dtype=np.float32)
    theta = freqs[:, None].astype(np.float32) * np.arange(L, dtype=np.float32)[None, :]
    cos32 = np.cos(theta).astype(np.float32)   # (32, L), freq j
    sin32 = np.sin(theta).astype(np.float32)
    cosd = np.ones((128, L), dtype=np.float32)
    sind = np.zeros((128, L), dtype=np.float32)
    for sq in range(2):   # quadrants 0,1 carry the 32 active freqs
        fr = slice(16 * sq, 16 * sq + 16)
        cosd[32 * sq:32 * sq + 16] = cos32[fr]
        cosd[32 * sq + 16:32 * sq + 32] = cos32[fr]
        sind[32 * sq:32 * sq + 16] = sin32[fr]
        sind[32 * sq + 16:32 * sq + 32] = -sin32[fr]
    return cosd, sind


def _make_in_maps(x, Wqkv, Wo, s):
    bf16 = mybir.dt.np(BF16)
    x = np.asarray(x, dtype=np.float32)
    Wqkv = np.asarray(Wqkv, dtype=np.float32)
    Wo = np.asarray(Wo, dtype=np.float32)
    s = np.asarray(s, dtype=np.float32)
    # pack to SBUF layout [partition, chunk, col]: xT[p, k, l] = x[l, k*128+p]
    xT = np.ascontiguousarray(
        x.reshape(L, NDK, 128).transpose(2, 1, 0)).astype(bf16)
    cosd, sind = _rope_tables()
    cosd = cosd.astype(bf16)
    sind = sind.astype(bf16)
    in_maps = []
    for c in range(N_CORES):
        g = c // 2
        wslc = np.concatenate([
            Wqkv[:, (2 * c) * HD:(2 * c) * HD + HD][:, ROPE_PERM],
            Wqkv[:, (2 * c + 1) * HD:(2 * c + 2) * HD][:, ROPE_PERM],
            Wqkv[:, 16 * HD + g * HD:16 * HD + (g + 1) * HD][:, ROPE_PERM],
            Wqkv[:, 20 * HD + g * HD:20 * HD + (g + 1) * HD],
        ], axis=1)
        wslc_p = np.ascontiguousarray(
            wslc.reshape(NDK, 128, 4 * HD).transpose(1, 0, 2)).astype(bf16)
        in_maps.append({
            "xT": xT,
            "wslc": wslc_p,
            "wo": np.ascontiguousarray(Wo[(2 * c) * HD:(2 * c + 2) * HD, :]).astype(bf16),
            "snk": np.ascontiguousarray(s[:, 2 * c:2 * c + 2]),
            "cosd": cosd,
            "sind": sind,
        })
    return in_maps


_CACHE = {}


def _get_exec():
    """Build the program once and return a cached jitted 8-core executor."""
    if "exec" in _CACHE:
        return _CACHE["exec"]

    import jax
    from jax.sharding import Mesh, PartitionSpec
    from jax.experimental.shard_map import shard_map
    from concourse.bass2jax import (_bass_exec_p, install_neuronx_cc_hook,
                                    partition_id_tensor)

    nc = _build_program()
    install_neuronx_cc_hook()

    partition_name = (nc.partition_id_tensor.name
                      if nc.partition_id_tensor else None)
    in_names, out_names, out_avals = [], [], []
    for alloc in nc.m.functions[0].allocations:
        if not isinstance(alloc, mybir.MemoryLocationSet):
            continue
        name = alloc.memorylocations[0].name
        if alloc.kind == "ExternalInput":
            if name != partition_name:
                in_names.append(name)
        elif alloc.kind == "ExternalOutput":
            out_names.append(name)
            out_avals.append(jax.core.ShapedArray(
                tuple(alloc.tensor_shape), mybir.dt.np(alloc.dtype)))
    n_params = len(in_names)
    all_names = in_names + out_names
    if partition_name is not None:
        all_names = all_names + [partition_name]

    def _body(*args):
        operands = list(args)
        if partition_name is not None:
            operands.append(partition_id_tensor())
        outs = _bass_exec_p.bind(
            *operands,
            out_avals=tuple(out_avals),
            in_names=tuple(all_names),
            out_names=tuple(out_names),
            lowering_input_output_aliases=(),
            sim_require_finite=True,
            sim_require_nnan=True,
            nc=nc,
        )
        return tuple(outs)

    devices = jax.devices()[:N_CORES]
    mesh = Mesh(np.asarray(devices), ("core",))
    n_outs = len(out_names)
    sharded = jax.jit(
        shard_map(_body, mesh=mesh,
                  in_specs=(PartitionSpec("core"),) * (n_params + n_outs),
                  out_specs=(PartitionSpec("core"),) * n_outs,
                  check_rep=False),
        keep_unused=True)

    state = {
        "sharded": sharded, "in_names": in_names, "out_names": out_names,
        "out_avals": out_avals, "mesh": mesh, "n_params": n_params,
        "nc": nc,
    }
    _CACHE["exec"] = state
    return state


def _run_cores(in_maps):
    ex = _get_exec()
    concat_in = [
        np.concatenate([np.asarray(m[name]) for m in in_maps], axis=0)
        for name in ex["in_names"]
    ]
    concat_zeros = [
        np.zeros((N_CORES * a.shape[0],) + tuple(a.shape[1:]), a.dtype)
        for a in ex["out_avals"]
    ]
    outs = ex["sharded"](*concat_in, *concat_zeros)
    name_to_i = {n: i for i, n in enumerate(ex["out_names"])}
    yi = name_to_i["y"]
    y_all = np.asarray(outs[yi]).reshape(N_CORES, L, D)
    return y_all


def kernel(x, Wqkv, Wo, s):
    in_maps = _make_in_maps(x, Wqkv, Wo, s)
    y_all = _run_cores(in_maps)
    out = y_all.sum(axis=0, dtype=np.float32)
    return out.reshape(1, L, D).astype(np.float32)


# revision 55
# speedup vs baseline: 1.3674x; 1.0964x over previous
"""Trainium2 Bass kernel for GroupedQueryAttention (inverted sliding-window mask + sink).

Full inputs in, full output out. Internally head-sharded across 8 NeuronCores:
core c handles q heads {2c, 2c+1} and kv head c//2, computes its partial
(x @ Wqkv_slice -> RoPE -> scores -> masked softmax w/ sink -> AV -> @ Wo_slice),
host sums the 8 partial outputs (the all-reduce).

v2: bf16 matmul inputs (host-cast), streamed x loads, post-exp multiplicative
masks on DVE, stream_shuffle RoPE, Wo tiles interleaved into next q-block.
"""

import os
import sys
from contextlib import ExitStack

sys.path.insert(0, "/opt/trn_rl_repo")

# jax must see the axon/neuron platform; a stray JAX_PLATFORMS=cpu would hide it.
if os.environ.get("JAX_PLATFORMS", "") == "cpu":
    os.environ["JAX_PLATFORMS"] = ""

import numpy as np

import concourse.bass as bass
import concourse.tile as tile
from concourse import bacc, mybir

F32 = mybir.dt.float32
BF16 = mybir.dt.bfloat16

LABELS = {}  # instruction name -> logical label (for trace analysis)


def _lbl(inst, label):
    try:
        LABELS[inst.ins.name] = label
    except Exception:
        pass
    return inst

N_CORES = 8
L = 2048
D = 2048
HD = 128
WINDOW = 1024
ROPE_BASE = 1024.0
SM_SCALE = 1.0 / float(np.sqrt(HD))

QB = 512          # q block (free dim of score tiles)
NQB = L // QB     # 4
NKT = L // HD     # 16 k tiles of 128
NDK = D // HD     # 16 contraction chunks for projections
NLB = L // QB     # 4 l-blocks for projection

# multiplicative-mask tiles are keyed by diff0 = q0 - k0 of the (k-tile, q-block)
MASK_DIFF0S = [0, -128, -256, -384, 640, 768, 896, 1024]
MASK_IDX = {d: i for i, d in enumerate(MASK_DIFF0S)}

# stream_shuffle permutes only within 32-partition quadrants, so q/k head
# dims are re-ordered (host side) to put each RoPE pair (d, d+64) 16 rows
# apart inside one quadrant: row 32s+i -> dim 16s+i, row 32s+16+i -> dim
# 64+16s+i (i<16). The shared permutation leaves q.k dot products unchanged.
SHUF16 = [(i + 16) % 32 for i in range(32)]
ROPE_PERM = np.array(
    [16 * s + i if i < 16 else 64 + 16 * s + (i - 16)
     for s in range(4) for i in range(32)])


def _classify(kt: int, qb: int):
    """masked band is 0 <= q-k <= WINDOW-1 (those entries get zeroed)."""
    d0 = QB * qb - HD * kt
    if 128 <= d0 <= 512:
        return "skip", None      # tile entirely inside the band -> contributes 0
    if d0 <= -512 or d0 >= 1152:
        return "full", None      # tile entirely outside the band -> no mask needed
    return "partial", MASK_IDX[d0]


def _build_program(dump=False):
    nc = bacc.Bacc("TRN2", target_bir_lowering=False, debug=False,
                   num_devices=N_CORES)
    dbg = {}
    if dump:
        for nm in ("dbg_q0", "dbg_q1", "dbg_k", "dbg_vT", "dbg_o0", "dbg_o1"):
            dbg[nm] = nc.dram_tensor(nm, [128, L], F32, kind="ExternalOutput").ap()
        dbg["dbg_v"] = nc.dram_tensor("dbg_v", [128, NKT * HD], F32,
                                      kind="ExternalOutput").ap()

    # host-packed to SBUF layout: [partition, chunk, col]
    xT_d = nc.dram_tensor("xT", [128, NDK, L], BF16, kind="ExternalInput").ap()
    wslc_d = nc.dram_tensor("wslc", [128, NDK, 4 * HD], BF16,
                            kind="ExternalInput").ap()
    wo_d = nc.dram_tensor("wo", [2 * HD, D], BF16, kind="ExternalInput").ap()
    snk_d = nc.dram_tensor("snk", [1, 2], F32, kind="ExternalInput").ap()
    cosd_d = nc.dram_tensor("cosd", [128, L], BF16, kind="ExternalInput").ap()
    sind_d = nc.dram_tensor("sind", [128, L], BF16, kind="ExternalInput").ap()
    # partial y ships bf16 (halves the dominant write traffic); host sums f32
    y_d = nc.dram_tensor("y", [L, D], BF16, kind="ExternalOutput").ap()

    with tile.TileContext(nc) as tc, ExitStack() as stk:
        persist = stk.enter_context(tc.tile_pool(name="persist", bufs=1))

        # ---- persistent SBUF tensors ----
        wslc_sb = persist.tile([128, NDK, 4 * HD], BF16, tag="wslc")
        wo_sb = persist.tile([128, 2, D], BF16, tag="wo")
        qT = [persist.tile([128, L], BF16, tag=f"qT{h}", name=f"qT{h}") for h in range(2)]
        kT = persist.tile([128, L], BF16, tag="kT")
        vT = persist.tile([128, L], BF16, tag="vT")
        v_sb = persist.tile([128, NKT, HD], BF16, tag="v")
        oT = [persist.tile([128, L], BF16, tag=f"oT{h}", name=f"oT{h}") for h in range(2)]
        cosd_sb = persist.tile([128, L], BF16, tag="cosd")
        sind_sb = persist.tile([128, L], BF16, tag="sind")
        masks = persist.tile([128, len(MASK_DIFF0S), QB], BF16, tag="masks")
        ident = persist.tile([128, 128], BF16, tag="ident")
        ones_bf = persist.tile([128, 1], BF16, tag="ones")
        snk_sb = persist.tile([1, 2], F32, tag="snk")
        exps_sb = persist.tile([1, 2], F32, tag="exps")

        # ---- weight loads ----
        # wslc split across SP and Act queues in small pieces so the
        # transfers interleave with the x-chunk stream on the DMA engines
        # (per-DMA queue hold is ~1.3us + transfer, so one queue can't feed
        # a chunk every 0.85us alone)
        for eng, pieces in ((nc.sync, ((0, 1), (2, 4), (6, 8), (10, 12), (14, 16))),
                            (nc.scalar, ((1, 2), (4, 6), (8, 10), (12, 14)))):
            for a, b in pieces:
                eng.dma_start(wslc_sb[:, a:b, :], wslc_d[:, a:b, :])
        nc.scalar.dma_start(snk_sb[:], snk_d[:])

        nc.vector.memset(ones_bf[:], 1.0)
        # exp of the two sink logits
        nc.scalar.activation(exps_sb[:], snk_sb[:], mybir.ActivationFunctionType.Exp)

        def build_masks():
            # multiplicative masks: 0 where 0 <= (q-k) <= WINDOW-1, else 1
            # (on Pool, emitted mid-phase-A: needed only at phase B)
            for i, d0 in enumerate(MASK_DIFF0S):
                m = masks[:, i, :]
                nc.gpsimd.memset(m, 1.0)
                if d0 <= 0:
                    # keep 1 where q-k < 0, i.e. kp - qf - d0 - 1 >= 0
                    nc.gpsimd.affine_select(
                        out=m, in_=m, compare_op=mybir.AluOpType.is_ge,
                        fill=0.0, base=-d0 - 1, channel_multiplier=1,
                        pattern=[[-1, QB]])
                else:
                    # keep 1 where q-k >= WINDOW, i.e. qf - kp + d0 - WINDOW >= 0
                    nc.gpsimd.affine_select(
                        out=m, in_=m, compare_op=mybir.AluOpType.is_ge,
                        fill=0.0, base=d0 - WINDOW, channel_multiplier=-1,
                        pattern=[[1, QB]])

        # ================= Phase A: QKV projection (transposed) =================
        # pT[c*128+r, l] = sum_d wslc[d, c*128+r] * x[l, d];  cols c = q0,q1,k,v
        col_dst = [qT[0], qT[1], kT, vT]

        xt_sb = persist.tile([128, NDK, L], BF16, tag="xt")

        # rope pool spans phases A and B: the last l-block's RoPE and
        # transposes are deferred into early attention
        rp = stk.enter_context(tc.tile_pool(name="rope", bufs=4))

        def rope_slice(t, lb):
            ls = slice(lb * QB, (lb + 1) * QB)
            partner = rp.tile([128, QB], BF16, tag="partner")
            nc.vector.stream_shuffle(partner[:], t[:, ls], SHUF16)
            tmp = rp.tile([128, QB], BF16, tag="ropetmp")
            nc.vector.tensor_mul(tmp[:], t[:, ls], cosd_sb[:, ls])
            nc.vector.tensor_mul(partner[:], partner[:], sind_sb[:, ls])
            nc.vector.tensor_add(t[:, ls], tmp[:], partner[:])

        def v_transposes(lb, pool, tag="vt"):
            pt = pool.tile([128, 4, HD], BF16, tag=tag)
            for j in range(4):
                t = 4 * lb + j
                _lbl(nc.tensor.transpose(
                    pt[:, j, :], vT[:, t * 128:(t + 1) * 128], ident[:]),
                    f"transp{t}")
            # DVE, not Act: Act must stay clear to start the exps promptly
            # (Pool can't read PSUM)
            nc.vector.tensor_copy(v_sb[:, 4 * lb:4 * lb + 4, :], pt[:])

        with tc.tile_pool(name="psA", bufs=6, space="PSUM") as psA, \
             tc.tile_pool(name="psT", bufs=2, space="PSUM") as psT:

            prev_v_lb = None
            for lb in range(NLB):
                ls = slice(lb * QB, (lb + 1) * QB)
                # x loads ride Pool's software DGE: the transfer is async, so
                # the Pool engine is only held ~1us per DMA. First block goes
                # per-chunk so PE starts as soon as possible.
                if lb == 0:
                    # Pool SWDGE issues ~1us apart vs PE consuming a chunk
                    # every ~0.85us: single chunks first, then pairs
                    for a, b in ((0, 1), (1, 2), (2, 4), (4, 6), (6, 8),
                                 (8, 10), (10, 13), (13, 16)):
                        nc.gpsimd.dma_start(
                            xt_sb[:, a:b, ls], xT_d[:, a:b, ls])
                else:
                    for g in range(4):
                        nc.gpsimd.dma_start(
                            xt_sb[:, 4 * g:4 * g + 4, ls],
                            xT_d[:, 4 * g:4 * g + 4, ls])
                # cos/sin slices arrive just before this block's RoPE; wo is
                # not needed until phase C — keep them off the early window
                nc.scalar.dma_start(cosd_sb[:, ls], cosd_d[:, ls])
                nc.scalar.dma_start(sind_sb[:, ls], sind_d[:, ls])
                if lb == 1:
                    for h in range(2):
                        nc.scalar.dma_start(wo_sb[:, h, :],
                                            wo_d[h * 128:(h + 1) * 128, :])
                if lb == 0:
                    # identity for PE transposes (needed from lb1 on)
                    nc.gpsimd.memset(ident[:], 0.0)
                    nc.gpsimd.affine_select(
                        out=ident[:], in_=ident[:],
                        compare_op=mybir.AluOpType.not_equal,
                        fill=1.0, base=0, channel_multiplier=1,
                        pattern=[[-1, 128]])
                if lb == NLB - 1:
                    build_masks()
                psums = [psA.tile([128, QB], F32, tag="proj",
                                  name=f"psproj{c}") for c in range(4)]
                for k in range(NDK):
                    for c in range(4):
                        _lbl(nc.tensor.matmul(
                            psums[c][:],
                            wslc_sb[:, k, c * 128:(c + 1) * 128],
                            xt_sb[:, k, ls],
                            start=(k == 0), stop=(k == NDK - 1)),
                            f"proj_lb{lb}_k{k}_c{c}")
                if prev_v_lb is not None:
                    v_transposes(prev_v_lb, psT)
                # copies psum -> bf16 SBUF; k,q0 on Act; q1,v on DVE (k first:
                # the first attention ldweights waits on kT's last write)
                nc.scalar.copy(kT[:, ls], psums[2][:])
                nc.scalar.copy(qT[0][:, ls], psums[0][:])
                nc.vector.tensor_copy(qT[1][:, ls], psums[1][:])
                nc.vector.tensor_copy(vT[:, ls], psums[3][:])
                # RoPE on this slice (DVE), overlapped with next lb's matmuls;
                # the last block's RoPE is deferred into attention (nothing in
                # the first groups reads those slices)
                if lb < NLB - 1:
                    rope_slice(kT, lb)
                    rope_slice(qT[0], lb)
                    rope_slice(qT[1], lb)
                prev_v_lb = lb

        # ============ Phase B+C: attention + output projection ============
        ycnt = [0]
        ystage = {}

        def make_emit_y(psY, sbY, act_share):
            def emit_y_tile(qt, nb):
                # one [128,512] y tile: 2 Wo matmuls + copy into a per-row
                # staging buffer; the whole [128,2048] row block ships as one
                # DMA (SP holds its SEQ for the full transfer; Pool's SWDGE
                # is async, so alternate).
                qts = slice(qt * 128, (qt + 1) * 128)
                ns = slice(nb * QB, (nb + 1) * QB)
                if nb == 0:
                    ystage[qt] = sbY.tile([128, D // QB, QB], BF16, tag="ysb",
                                          name=f"ystage{qt}")
                psum_y = psY.tile([128, QB], F32, tag="y")
                for h in range(2):
                    _lbl(nc.tensor.matmul(
                        psum_y[:],
                        oT[h][:, qts],
                        wo_sb[:, h, ns],
                        start=(h == 0), stop=(h == 1)),
                        f"y_qt{qt}_nb{nb}_h{h}")
                # copies mostly on DVE; every 4th on Act (DVE runs slightly
                # over in the mask-heavy late groups)
                if ycnt[0] % (2 if act_share else 4) == 1:
                    nc.scalar.copy(ystage[qt][:, nb, :], psum_y[:])
                else:
                    nc.vector.tensor_copy(ystage[qt][:, nb, :], psum_y[:])
                ycnt[0] += 1
                if nb == D // QB - 1:
                    # Pool SWDGE (async) for in-group rows; final-drain rows
                    # split SP/Pool so the last flush overlaps
                    if act_share:
                        nc.sync.dma_start(y_d[qts, 0:D // 2],
                                          ystage[qt][:, 0:2, :])
                        nc.gpsimd.dma_start(y_d[qts, D // 2:D],
                                            ystage[qt][:, 2:4, :])
                    else:
                        nc.gpsimd.dma_start(y_d[qts, :], ystage[qt][:])
                    del ystage[qt]
            return emit_y_tile

        with tc.tile_pool(name="psS", bufs=4, space="PSUM") as psS, \
             tc.tile_pool(name="psO", bufs=2, space="PSUM") as psO, \
             tc.tile_pool(name="psD", bufs=1, space="PSUM") as psD, \
             tc.tile_pool(name="psY", bufs=1, space="PSUM") as psY, \
             tc.tile_pool(name="epool", bufs=8) as epool, \
             tc.tile_pool(name="sbB", bufs=4) as sbB, \
             tc.tile_pool(name="sbY", bufs=4) as sbY:

            emit_y_tile = make_emit_y(psY, sbY, act_share=False)

            pending_y = []
            LAG = 3  # tiles between score emission and its den/AV, hiding exp
            for qb in range(NQB):
                qs = slice(qb * QB, (qb + 1) * QB)
                for h in range(2):
                    acts = [(kt, _classify(kt, qb)) for kt in range(NKT)]
                    acts = [(kt, c, mi) for kt, (c, mi) in acts if c != "skip"]
                    # full tiles first: partial tiles' den/AV depend on DVE
                    # mask-muls, so give DVE the whole group to produce them
                    acts = ([a for a in acts if a[1] == "full"]
                            + [a for a in acts if a[1] == "partial"])
                    n_act = len(acts)
                    first_group = (qb == 0 and h == 0)
                    psum_o = psO.tile([128, QB], F32, tag="o")
                    psum_den = psD.tile([1, QB], F32, tag="den")
                    e_use = [None] * n_act
                    for i in range(n_act + LAG):
                        if i < n_act:
                            kt, cls, mi = acts[i]
                            psum_s = psS.tile([128, QB], F32, tag="s")
                            _lbl(nc.tensor.matmul(
                                psum_s[:],
                                kT[:, kt * 128:(kt + 1) * 128],
                                qT[h][:, qs],
                                start=True, stop=True),
                                f"score_h{h}_qb{qb}_kt{kt}")
                            e_sb = epool.tile([128, QB], BF16, tag="e")
                            nc.scalar.activation(
                                e_sb[:], psum_s[:],
                                mybir.ActivationFunctionType.Exp,
                                scale=SM_SCALE)
                            if cls == "partial":
                                e_m = epool.tile([128, QB], BF16, tag="em")
                                nc.vector.tensor_mul(
                                    e_m[:], e_sb[:], masks[:, mi, :])
                                e_use[i] = e_m
                            else:
                                e_use[i] = e_sb
                        if first_group:
                            # deferred last-l-block prep, overlapped with the
                            # first group's score/exp pipeline
                            if i == 0:
                                rope_slice(kT, NLB - 1)
                            elif i == 1:
                                rope_slice(qT[0], NLB - 1)
                            elif i == 2:
                                rope_slice(qT[1], NLB - 1)
                            elif i == 4:
                                # borrows a score-pool buffer (same bank size)
                                v_transposes(NLB - 1, psS, tag="s")
                        j = i - LAG
                        if 0 <= j < n_act:
                            ktj = acts[j][0]
                            _lbl(nc.tensor.matmul(
                                psum_den[:], ones_bf[:],
                                e_use[j][:],
                                start=(j == 0), stop=(j == n_act - 1)),
                                f"den_h{h}_qb{qb}_kt{ktj}")
                            _lbl(nc.tensor.matmul(
                                psum_o[:], v_sb[:, ktj, :],
                                e_use[j][:],
                                start=(j == 0), stop=(j == n_act - 1)),
                                f"av_h{h}_qb{qb}_kt{ktj}")
                            e_use[j] = None
                        # interleave one deferred y tile of the previous q block
                        if pending_y and i >= 2:
                            emit_y_tile(*pending_y.pop(0))
                    den_sb = sbB.tile([1, QB], F32, tag="densb")
                    nc.scalar.activation(
                        den_sb[:], psum_den[:],
                        mybir.ActivationFunctionType.Identity,
                        bias=exps_sb[0:1, h:h + 1])
                    r_sb = sbB.tile([1, QB], F32, tag="rsb")
                    nc.vector.reciprocal(r_sb[:], den_sb[:])
                    rb = sbB.tile([128, QB], F32, tag="rb")
                    nc.gpsimd.partition_broadcast(rb[:], r_sb[:])
                    nc.vector.tensor_mul(oT[h][:, qs], psum_o[:], rb[:])
                pending_y.extend(
                    (qb * (QB // 128) + j, nb)
                    for j in range(QB // 128) for nb in range(D // QB))

        # final-qb y drain: attention pools are done, so rebuild with deep
        # buffering and let the copies use both Act and DVE
        with tc.tile_pool(name="psY2", bufs=4, space="PSUM") as psY2, \
             tc.tile_pool(name="sbY2", bufs=4) as sbY2:
            emit_y_tile = make_emit_y(psY2, sbY2, act_share=True)
            while pending_y:
                emit_y_tile(*pending_y.pop(0))

        if dump:
            with tc.tile_pool(name="dbgp", bufs=2) as dbgp:
                for nm, t in (("dbg_q0", qT[0]), ("dbg_q1", qT[1]),
                              ("dbg_k", kT), ("dbg_vT", vT),
                              ("dbg_o0", oT[0]), ("dbg_o1", oT[1])):
                    f = dbgp.tile([128, L], F32, tag="dbgf", name=f"f{nm}")
                    nc.scalar.copy(f[:], t[:])
                    nc.sync.dma_start(dbg[nm], f[:])
                fv = dbgp.tile([128, NKT, HD], F32, tag="dbgf", name="fv")
                nc.scalar.copy(fv[:], v_sb[:])
                nc.sync.dma_start(dbg["dbg_v"], fv[:])

    nc.compile()
    return nc


def _rope_tables():
    """cos/sin tables in the permuted row order (see ROPE_PERM)."""
    freqs = (1.0 / ROPE_BASE) ** np.linspace(0.0, 1.0, num=HD // 4,
                                             dtype=np.float32)
    theta = freqs[:, None].astype(np.float32) * np.arange(L, dtype=np.float32)[None, :]
    cos32 = np.cos(theta).astype(np.float32)   # (32, L), freq j
    sin32 = np.sin(theta).astype(np.float32)
    cosd = np.ones((128, L), dtype=np.float32)
    sind = np.zeros((128, L), dtype=np.float32)
    for sq in range(2):   # quadrants 0,1 carry the 32 active freqs
        fr = slice(16 * sq, 16 * sq + 16)
        cosd[32 * sq:32 * sq + 16] = cos32[fr]
        cosd[32 * sq + 16:32 * sq + 32] = cos32[fr]
        sind[32 * sq:32 * sq + 16] = sin32[fr]
        sind[32 * sq + 16:32 * sq + 32] = -sin32[fr]
    return cosd, sind


def _make_in_maps(x, Wqkv, Wo, s):
    bf16 = mybir.dt.np(BF16)
    x = np.asarray(x, dtype=np.float32)
    Wqkv = np.asarray(Wqkv, dtype=np.float32)
    Wo = np.asarray(Wo, dtype=np.float32)
    s = np.asarray(s, dtype=np.float32)
    # pack to SBUF layout [partition, chunk, col]: xT[p, k, l] = x[l, k*128+p]
    xT = np.ascontiguousarray(
        x.reshape(L, NDK, 128).transpose(2, 1, 0)).astype(bf16)
    cosd, sind = _rope_tables()
    cosd = cosd.astype(bf16)
    sind = sind.astype(bf16)
    in_maps = []
    for c in range(N_CORES):
        g = c // 2
        wslc = np.concatenate([
            Wqkv[:, (2 * c) * HD:(2 * c) * HD + HD][:, ROPE_PERM],
            Wqkv[:, (2 * c + 1) * HD:(2 * c + 2) * HD][:, ROPE_PERM],
            Wqkv[:, 16 * HD + g * HD:16 * HD + (g + 1) * HD][:, ROPE_PERM],
            Wqkv[:, 20 * HD + g * HD:20 * HD + (g + 1) * HD],
        ], axis=1)
        wslc_p = np.ascontiguousarray(
            wslc.reshape(NDK, 128, 4 * HD).transpose(1, 0, 2)).astype(bf16)
        in_maps.append({
            "xT": xT,
            "wslc": wslc_p,
            "wo": np.ascontiguousarray(Wo[(2 * c) * HD:(2 * c + 2) * HD, :]).astype(bf16),
            "snk": np.ascontiguousarray(s[:, 2 * c:2 * c + 2]),
            "cosd": cosd,
            "sind": sind,
        })
    return in_maps


_CACHE = {}


def _get_exec():
    """Build the program once and return a cached jitted 8-core executor."""
    if "exec" in _CACHE:
        return _CACHE["exec"]

    import jax
    from jax.sharding import Mesh, PartitionSpec
    from jax.experimental.shard_map import shard_map
    from concourse.bass2jax import (_bass_exec_p, install_neuronx_cc_hook,
                                    partition_id_tensor)

    nc = _build_program()
    install_neuronx_cc_hook()

    partition_name = (nc.partition_id_tensor.name
                      if nc.partition_id_tensor else None)
    in_names, out_names, out_avals = [], [], []
    for alloc in nc.m.functions[0].allocations:
        if not isinstance(alloc, mybir.MemoryLocationSet):
            continue
        name = alloc.memorylocations[0].name
        if alloc.kind == "ExternalInput":
            if name != partition_name:
                in_names.append(name)
        elif alloc.kind == "ExternalOutput":
            out_names.append(name)
            out_avals.append(jax.core.ShapedArray(
                tuple(alloc.tensor_shape), mybir.dt.np(alloc.dtype)))
    n_params = len(in_names)
    all_names = in_names + out_names
    if partition_name is not None:
        all_names = all_names + [partition_name]

    def _body(*args):
        operands = list(args)
        if partition_name is not None:
            operands.append(partition_id_tensor())
        outs = _bass_exec_p.bind(
            *operands,
            out_avals=tuple(out_avals),
            in_names=tuple(all_names),
            out_names=tuple(out_names),
            lowering_input_output_aliases=(),
            sim_require_finite=True,
            sim_require_nnan=True,
            nc=nc,
        )
        return tuple(outs)

    devices = jax.devices()[:N_CORES]
    mesh = Mesh(np.asarray(devices), ("core",))
    n_outs = len(out_names)
    sharded = jax.jit(
        shard_map(_body, mesh=mesh,
                  in_specs=(PartitionSpec("core"),) * (n_params + n_outs),
                  out_specs=(PartitionSpec("core"),) * n_outs,
                  check_rep=False),
        keep_unused=True)

    state = {
        "sharded": sharded, "in_names": in_names, "out_names": out_names,
        "out_avals": out_avals, "mesh": mesh, "n_params": n_params,
        "nc": nc,
    }
    _CACHE["exec"] = state
    return state


def _run_cores(in_maps):
    ex = _get_exec()
    concat_in = [
        np.concatenate([np.asarray(m[name]) for m in in_maps], axis=0)
        for name in ex["in_names"]
    ]
    concat_zeros = [
        np.zeros((N_CORES * a.shape[0],) + tuple(a.shape[1:]), a.dtype)
        for a in ex["out_avals"]
    ]
    outs = ex["sharded"](*concat_in, *concat_zeros)
    name_to_i = {n: i for i, n in enumerate(ex["out_names"])}
    yi = name_to_i["y"]
    y_all = np.asarray(outs[yi]).astype(np.float32).reshape(N_CORES, L, D)
    return y_all


def kernel(x, Wqkv, Wo, s):
    in_maps = _make_in_maps(x, Wqkv, Wo, s)
    y_all = _run_cores(in_maps)
    out = y_all.sum(axis=0, dtype=np.float32)
    return out.reshape(1, L, D).astype(np.float32)


# revision 60
# speedup vs baseline: 1.4036x; 1.0265x over previous
"""Trainium2 Bass kernel for GroupedQueryAttention (inverted sliding-window mask + sink).

Full inputs in, full output out. Internally head-sharded across 8 NeuronCores:
core c handles q heads {2c, 2c+1} and kv head c//2, computes its partial
(x @ Wqkv_slice -> RoPE -> scores -> masked softmax w/ sink -> AV -> @ Wo_slice),
host sums the 8 partial outputs (the all-reduce).

v2: bf16 matmul inputs (host-cast), streamed x loads, post-exp multiplicative
masks on DVE, stream_shuffle RoPE, Wo tiles interleaved into next q-block.
"""

import os
import sys
from contextlib import ExitStack

sys.path.insert(0, "/opt/trn_rl_repo")

# jax must see the axon/neuron platform; a stray JAX_PLATFORMS=cpu would hide it.
if os.environ.get("JAX_PLATFORMS", "") == "cpu":
    os.environ["JAX_PLATFORMS"] = ""

import numpy as np

import concourse.bass as bass
import concourse.tile as tile
from concourse import bacc, mybir

F32 = mybir.dt.float32
BF16 = mybir.dt.bfloat16

LABELS = {}  # instruction name -> logical label (for trace analysis)


def _lbl(inst, label):
    try:
        LABELS[inst.ins.name] = label
    except Exception:
        pass
    return inst

N_CORES = 8
L = 2048
D = 2048
HD = 128
WINDOW = 1024
ROPE_BASE = 1024.0
SM_SCALE = 1.0 / float(np.sqrt(HD))

QB = 512          # q block (free dim of score tiles)
NQB = L // QB     # 4
NKT = L // HD     # 16 k tiles of 128
NDK = D // HD     # 16 contraction chunks for projections
NLB = L // QB     # 4 l-blocks for projection

# multiplicative-mask tiles are keyed by diff0 = q0 - k0 of the (k-tile, q-block)
MASK_DIFF0S = [0, -128, -256, -384, 640, 768, 896, 1024]
MASK_IDX = {d: i for i, d in enumerate(MASK_DIFF0S)}

# stream_shuffle permutes only within 32-partition quadrants, so q/k head
# dims are re-ordered (host side) to put each RoPE pair (d, d+64) 16 rows
# apart inside one quadrant: row 32s+i -> dim 16s+i, row 32s+16+i -> dim
# 64+16s+i (i<16). The shared permutation leaves q.k dot products unchanged.
SHUF16 = [(i + 16) % 32 for i in range(32)]
ROPE_PERM = np.array(
    [16 * s + i if i < 16 else 64 + 16 * s + (i - 16)
     for s in range(4) for i in range(32)])


def _classify(kt: int, qb: int):
    """masked band is 0 <= q-k <= WINDOW-1 (those entries get zeroed)."""
    d0 = QB * qb - HD * kt
    if 128 <= d0 <= 512:
        return "skip", None      # tile entirely inside the band -> contributes 0
    if d0 <= -512 or d0 >= 1152:
        return "full", None      # tile entirely outside the band -> no mask needed
    return "partial", MASK_IDX[d0]


def _build_program(dump=False):
    nc = bacc.Bacc("TRN2", target_bir_lowering=False, debug=False,
                   num_devices=N_CORES)
    dbg = {}
    if dump:
        for nm in ("dbg_q0", "dbg_q1", "dbg_k", "dbg_vT", "dbg_o0", "dbg_o1"):
            dbg[nm] = nc.dram_tensor(nm, [128, L], F32, kind="ExternalOutput").ap()
        dbg["dbg_v"] = nc.dram_tensor("dbg_v", [128, NKT * HD], F32,
                                      kind="ExternalOutput").ap()

    # host-packed to SBUF layout: [partition, chunk, col]
    xT_d = nc.dram_tensor("xT", [128, NDK, L], BF16, kind="ExternalInput").ap()
    wslc_d = nc.dram_tensor("wslc", [128, NDK, 4 * HD], BF16,
                            kind="ExternalInput").ap()
    wo_d = nc.dram_tensor("wo", [2 * HD, D], BF16, kind="ExternalInput").ap()
    snk_d = nc.dram_tensor("snk", [1, 2], F32, kind="ExternalInput").ap()
    cosd_d = nc.dram_tensor("cosd", [128, L], BF16, kind="ExternalInput").ap()
    sind_d = nc.dram_tensor("sind", [128, L], BF16, kind="ExternalInput").ap()
    # partial y ships bf16 (halves the dominant write traffic); host sums f32
    y_d = nc.dram_tensor("y", [L, D], BF16, kind="ExternalOutput").ap()

    with tile.TileContext(nc) as tc, ExitStack() as stk:
        persist = stk.enter_context(tc.tile_pool(name="persist", bufs=1))

        # ---- persistent SBUF tensors ----
        wslc_sb = persist.tile([128, NDK, 4 * HD], BF16, tag="wslc")
        wo_sb = persist.tile([128, 2, D], BF16, tag="wo")
        qT = [persist.tile([128, L], BF16, tag=f"qT{h}", name=f"qT{h}") for h in range(2)]
        kT = persist.tile([128, L], BF16, tag="kT")
        vT = persist.tile([128, L], BF16, tag="vT")
        v_sb = persist.tile([128, NKT, HD], BF16, tag="v")
        oT = [persist.tile([128, L], BF16, tag=f"oT{h}", name=f"oT{h}") for h in range(2)]
        cosd_sb = persist.tile([128, L], BF16, tag="cosd")
        sind_sb = persist.tile([128, L], BF16, tag="sind")
        masks = persist.tile([128, len(MASK_DIFF0S), QB], BF16, tag="masks")
        ident = persist.tile([128, 128], BF16, tag="ident")
        ones_bf = persist.tile([128, 1], BF16, tag="ones")
        snk_sb = persist.tile([1, 2], F32, tag="snk")
        exps_sb = persist.tile([1, 2], F32, tag="exps")

        # ---- weight loads ----
        # wslc split across SP and Act queues in small pieces so the
        # transfers interleave with the x-chunk stream on the DMA engines
        # (per-DMA queue hold is ~1.3us + transfer, so one queue can't feed
        # a chunk every 0.85us alone)
        for eng, pieces in ((nc.sync, ((0, 1), (2, 4), (6, 8), (10, 12), (14, 16))),
                            (nc.scalar, ((1, 2), (4, 6), (8, 10), (12, 14)))):
            for a, b in pieces:
                eng.dma_start(wslc_sb[:, a:b, :], wslc_d[:, a:b, :])
        nc.scalar.dma_start(snk_sb[:], snk_d[:])

        nc.vector.memset(ones_bf[:], 1.0)
        # exp of the two sink logits
        nc.scalar.activation(exps_sb[:], snk_sb[:], mybir.ActivationFunctionType.Exp)

        def build_masks():
            # multiplicative masks: 0 where 0 <= (q-k) <= WINDOW-1, else 1
            # (on Pool, emitted mid-phase-A: needed only at phase B)
            for i, d0 in enumerate(MASK_DIFF0S):
                m = masks[:, i, :]
                nc.gpsimd.memset(m, 1.0)
                if d0 <= 0:
                    # keep 1 where q-k < 0, i.e. kp - qf - d0 - 1 >= 0
                    nc.gpsimd.affine_select(
                        out=m, in_=m, compare_op=mybir.AluOpType.is_ge,
                        fill=0.0, base=-d0 - 1, channel_multiplier=1,
                        pattern=[[-1, QB]])
                else:
                    # keep 1 where q-k >= WINDOW, i.e. qf - kp + d0 - WINDOW >= 0
                    nc.gpsimd.affine_select(
                        out=m, in_=m, compare_op=mybir.AluOpType.is_ge,
                        fill=0.0, base=d0 - WINDOW, channel_multiplier=-1,
                        pattern=[[1, QB]])

        # ================= Phase A: QKV projection (transposed) =================
        # pT[c*128+r, l] = sum_d wslc[d, c*128+r] * x[l, d];  cols c = q0,q1,k,v
        col_dst = [qT[0], qT[1], kT, vT]

        xt_sb = persist.tile([128, NDK, L], BF16, tag="xt")

        # rope pool spans phases A and B: the last l-block's RoPE and
        # transposes are deferred into early attention
        rp = stk.enter_context(tc.tile_pool(name="rope", bufs=4))

        def rope_slice(t, lb):
            ls = slice(lb * QB, (lb + 1) * QB)
            partner = rp.tile([128, QB], BF16, tag="partner")
            nc.vector.stream_shuffle(partner[:], t[:, ls], SHUF16)
            tmp = rp.tile([128, QB], BF16, tag="ropetmp")
            nc.vector.tensor_mul(tmp[:], t[:, ls], cosd_sb[:, ls])
            nc.vector.tensor_mul(partner[:], partner[:], sind_sb[:, ls])
            nc.vector.tensor_add(t[:, ls], tmp[:], partner[:])

        def v_transposes(lb, pool, tag="vt"):
            pt = pool.tile([128, 4, HD], BF16, tag=tag)
            for j in range(4):
                t = 4 * lb + j
                _lbl(nc.tensor.transpose(
                    pt[:, j, :], vT[:, t * 128:(t + 1) * 128], ident[:]),
                    f"transp{t}")
            # DVE, not Act: Act must stay clear to start the exps promptly
            # (Pool can't read PSUM)
            nc.vector.tensor_copy(v_sb[:, 4 * lb:4 * lb + 4, :], pt[:])

        with tc.tile_pool(name="psA", bufs=6, space="PSUM") as psA, \
             tc.tile_pool(name="psT", bufs=2, space="PSUM") as psT:

            prev_v_lb = None
            for lb in range(NLB):
                ls = slice(lb * QB, (lb + 1) * QB)
                # x loads ride Pool's software DGE: the transfer is async, so
                # the Pool engine is only held ~1us per DMA. First block goes
                # per-chunk so PE starts as soon as possible.
                if lb == 0:
                    # Pool SWDGE issues ~1us apart vs PE consuming a chunk
                    # every ~0.85us: single chunks first, then pairs
                    for a, b in ((0, 1), (1, 2), (2, 4), (4, 6), (6, 8),
                                 (8, 10), (10, 13), (13, 16)):
                        nc.gpsimd.dma_start(
                            xt_sb[:, a:b, ls], xT_d[:, a:b, ls])
                else:
                    for g in range(4):
                        nc.gpsimd.dma_start(
                            xt_sb[:, 4 * g:4 * g + 4, ls],
                            xT_d[:, 4 * g:4 * g + 4, ls])
                # cos/sin slices arrive just before this block's RoPE; wo is
                # not needed until phase C — keep them off the early window
                nc.scalar.dma_start(cosd_sb[:, ls], cosd_d[:, ls])
                nc.scalar.dma_start(sind_sb[:, ls], sind_d[:, ls])
                if lb == 1:
                    for h in range(2):
                        nc.scalar.dma_start(wo_sb[:, h, :],
                                            wo_d[h * 128:(h + 1) * 128, :])
                if lb == 0:
                    # identity for PE transposes (needed from lb1 on)
                    nc.gpsimd.memset(ident[:], 0.0)
                    nc.gpsimd.affine_select(
                        out=ident[:], in_=ident[:],
                        compare_op=mybir.AluOpType.not_equal,
                        fill=1.0, base=0, channel_multiplier=1,
                        pattern=[[-1, 128]])
                if lb == NLB - 1:
                    build_masks()
                psums = [psA.tile([128, QB], F32, tag="proj",
                                  name=f"psproj{c}") for c in range(4)]
                for k in range(NDK):
                    for c in range(4):
                        _lbl(nc.tensor.matmul(
                            psums[c][:],
                            wslc_sb[:, k, c * 128:(c + 1) * 128],
                            xt_sb[:, k, ls],
                            start=(k == 0), stop=(k == NDK - 1)),
                            f"proj_lb{lb}_k{k}_c{c}")
                if prev_v_lb is not None:
                    v_transposes(prev_v_lb, psT)
                # copies psum -> bf16 SBUF; k,q0 on Act; q1,v on DVE (k first:
                # the first attention ldweights waits on kT's last write)
                nc.scalar.copy(kT[:, ls], psums[2][:])
                nc.scalar.copy(qT[0][:, ls], psums[0][:])
                nc.vector.tensor_copy(qT[1][:, ls], psums[1][:])
                nc.vector.tensor_copy(vT[:, ls], psums[3][:])
                # RoPE on this slice (DVE), overlapped with next lb's matmuls;
                # the last block's RoPE is deferred into attention (nothing in
                # the first groups reads those slices)
                if lb < NLB - 1:
                    rope_slice(kT, lb)
                    rope_slice(qT[0], lb)
                    rope_slice(qT[1], lb)
                prev_v_lb = lb

        # ============ Phase B+C: attention + output projection ============
        ycnt = [0]
        ystage = {}

        def make_emit_y(psY, sbY, act_share):
            def emit_y_tile(qt, nb):
                # one [128,512] y tile: 2 Wo matmuls + copy into a per-row
                # staging buffer; the whole [128,2048] row block ships as one
                # DMA (SP holds its SEQ for the full transfer; Pool's SWDGE
                # is async, so alternate).
                qts = slice(qt * 128, (qt + 1) * 128)
                ns = slice(nb * QB, (nb + 1) * QB)
                if nb == 0:
                    ystage[qt] = sbY.tile([128, D // QB, QB], BF16, tag="ysb",
                                          name=f"ystage{qt}")
                psum_y = psY.tile([128, QB], F32, tag="y")
                for h in range(2):
                    _lbl(nc.tensor.matmul(
                        psum_y[:],
                        oT[h][:, qts],
                        wo_sb[:, h, ns],
                        start=(h == 0), stop=(h == 1)),
                        f"y_qt{qt}_nb{nb}_h{h}")
                # copies mostly on DVE; every 4th on Act (DVE runs slightly
                # over in the mask-heavy late groups)
                if ycnt[0] % (2 if act_share else 4) == 1:
                    nc.scalar.copy(ystage[qt][:, nb, :], psum_y[:])
                else:
                    nc.vector.tensor_copy(ystage[qt][:, nb, :], psum_y[:])
                ycnt[0] += 1
                # Pool SWDGE (async) for in-group rows; final-drain rows are
                # split across SP/Act/Pool queues piece-by-piece so the last
                # flush is a single small racing transfer per queue
                if act_share:
                    if nb == 1:
                        nc.sync.dma_start(y_d[qts, 0:D // 2],
                                          ystage[qt][:, 0:2, :])
                    elif nb == 2:
                        nc.sync.dma_start(y_d[qts, D // 2:3 * D // 4],
                                          ystage[qt][:, 2, :])
                    elif nb == 3:
                        nc.gpsimd.dma_start(y_d[qts, 3 * D // 4:D],
                                            ystage[qt][:, 3, :])
                        del ystage[qt]
                elif nb == D // QB - 1:
                    nc.gpsimd.dma_start(y_d[qts, :], ystage[qt][:])
                    del ystage[qt]
            return emit_y_tile

        with tc.tile_pool(name="psS", bufs=4, space="PSUM") as psS, \
             tc.tile_pool(name="psO", bufs=2, space="PSUM") as psO, \
             tc.tile_pool(name="psD", bufs=1, space="PSUM") as psD, \
             tc.tile_pool(name="psY", bufs=1, space="PSUM") as psY, \
             tc.tile_pool(name="epool", bufs=8) as epool, \
             tc.tile_pool(name="sbB", bufs=4) as sbB, \
             tc.tile_pool(name="sbY", bufs=4) as sbY:

            emit_y_tile = make_emit_y(psY, sbY, act_share=False)

            pending_y = []
            LAG = 3  # tiles between score emission and its den/AV, hiding exp
            for qb in range(NQB):
                qs = slice(qb * QB, (qb + 1) * QB)
                for h in range(2):
                    acts = [(kt, _classify(kt, qb)) for kt in range(NKT)]
                    acts = [(kt, c, mi) for kt, (c, mi) in acts if c != "skip"]
                    # full tiles first: partial tiles' den/AV depend on DVE
                    # mask-muls, so give DVE the whole group to produce them
                    acts = ([a for a in acts if a[1] == "full"]
                            + [a for a in acts if a[1] == "partial"])
                    n_act = len(acts)
                    first_group = (qb == 0 and h == 0)
                    psum_o = psO.tile([128, QB], F32, tag="o")
                    psum_den = psD.tile([1, QB], F32, tag="den")
                    e_use = [None] * n_act
                    for i in range(n_act + LAG):
                        if i < n_act:
                            kt, cls, mi = acts[i]
                            psum_s = psS.tile([128, QB], F32, tag="s")
                            _lbl(nc.tensor.matmul(
                                psum_s[:],
                                kT[:, kt * 128:(kt + 1) * 128],
                                qT[h][:, qs],
                                start=True, stop=True),
                                f"score_h{h}_qb{qb}_kt{kt}")
                            e_sb = epool.tile([128, QB], BF16, tag="e")
                            nc.scalar.activation(
                                e_sb[:], psum_s[:],
                                mybir.ActivationFunctionType.Exp,
                                scale=SM_SCALE)
                            if cls == "partial":
                                e_m = epool.tile([128, QB], BF16, tag="em")
                                nc.vector.tensor_mul(
                                    e_m[:], e_sb[:], masks[:, mi, :])
                                e_use[i] = e_m
                            else:
                                e_use[i] = e_sb
                        if first_group:
                            # deferred last-l-block prep, overlapped with the
                            # first group's score/exp pipeline
                            if i == 0:
                                rope_slice(kT, NLB - 1)
                            elif i == 1:
                                rope_slice(qT[0], NLB - 1)
                            elif i == 2:
                                rope_slice(qT[1], NLB - 1)
                            elif i == 4:
                                # borrows a score-pool buffer (same bank size)
                                v_transposes(NLB - 1, psS, tag="s")
                        j = i - LAG
                        if 0 <= j < n_act:
                            ktj = acts[j][0]
                            _lbl(nc.tensor.matmul(
                                psum_den[:], ones_bf[:],
                                e_use[j][:],
                                start=(j == 0), stop=(j == n_act - 1)),
                                f"den_h{h}_qb{qb}_kt{ktj}")
                            _lbl(nc.tensor.matmul(
                                psum_o[:], v_sb[:, ktj, :],
                                e_use[j][:],
                                start=(j == 0), stop=(j == n_act - 1)),
                                f"av_h{h}_qb{qb}_kt{ktj}")
                            e_use[j] = None
                        # interleave one deferred y tile of the previous q block
                        if pending_y and i >= 2:
                            emit_y_tile(*pending_y.pop(0))
                    den_sb = sbB.tile([1, QB], F32, tag="densb")
                    nc.scalar.activation(
                        den_sb[:], psum_den[:],
                        mybir.ActivationFunctionType.Identity,
                        bias=exps_sb[0:1, h:h + 1])
                    r_sb = sbB.tile([1, QB], F32, tag="rsb")
                    nc.vector.reciprocal(r_sb[:], den_sb[:])
                    rb = sbB.tile([128, QB], F32, tag="rb")
                    nc.gpsimd.partition_broadcast(rb[:], r_sb[:])
                    nc.vector.tensor_mul(oT[h][:, qs], psum_o[:], rb[:])
                pending_y.extend(
                    (qb * (QB // 128) + j, nb)
                    for j in range(QB // 128) for nb in range(D // QB))

        # final-qb y drain: attention pools are done, so rebuild with deep
        # buffering and let the copies use both Act and DVE
        with tc.tile_pool(name="psY2", bufs=4, space="PSUM") as psY2, \
             tc.tile_pool(name="sbY2", bufs=4) as sbY2:
            emit_y_tile = make_emit_y(psY2, sbY2, act_share=True)
            while pending_y:
                emit_y_tile(*pending_y.pop(0))

        if dump:
            with tc.tile_pool(name="dbgp", bufs=2) as dbgp:
                for nm, t in (("dbg_q0", qT[0]), ("dbg_q1", qT[1]),
                              ("dbg_k", kT), ("dbg_vT", vT),
                              ("dbg_o0", oT[0]), ("dbg_o1", oT[1])):
                    f = dbgp.tile([128, L], F32, tag="dbgf", name=f"f{nm}")
                    nc.scalar.copy(f[:], t[:])
                    nc.sync.dma_start(dbg[nm], f[:])
                fv = dbgp.tile([128, NKT, HD], F32, tag="dbgf", name="fv")
                nc.scalar.copy(fv[:], v_sb[:])
                nc.sync.dma_start(dbg["dbg_v"], fv[:])

    nc.compile()
    return nc


def _rope_tables():
    """cos/sin tables in the permuted row order (see ROPE_PERM)."""
    freqs = (1.0 / ROPE_BASE) ** np.linspace(0.0, 1.0, num=HD // 4,
                                             dtype=np.float32)
    theta = freqs[:, None].astype(np.float32) * np.arange(L, dtype=np.float32)[None, :]
    cos32 = np.cos(theta).astype(np.float32)   # (32, L), freq j
    sin32 = np.sin(theta).astype(np.float32)
    cosd = np.ones((128, L), dtype=np.float32)
    sind = np.zeros((128, L), dtype=np.float32)
    for sq in range(2):   # quadrants 0,1 carry the 32 active freqs
        fr = slice(16 * sq, 16 * sq + 16)
        cosd[32 * sq:32 * sq + 16] = cos32[fr]
        cosd[32 * sq + 16:32 * sq + 32] = cos32[fr]
        sind[32 * sq:32 * sq + 16] = sin32[fr]
        sind[32 * sq + 16:32 * sq + 32] = -sin32[fr]
    return cosd, sind


def _make_in_maps(x, Wqkv, Wo, s):
    bf16 = mybir.dt.np(BF16)
    x = np.asarray(x, dtype=np.float32)
    Wqkv = np.asarray(Wqkv, dtype=np.float32)
    Wo = np.asarray(Wo, dtype=np.float32)
    s = np.asarray(s, dtype=np.float32)
    # pack to SBUF layout [partition, chunk, col]: xT[p, k, l] = x[l, k*128+p]
    xT = np.ascontiguousarray(
        x.reshape(L, NDK, 128).transpose(2, 1, 0)).astype(bf16)
    cosd, sind = _rope_tables()
    cosd = cosd.astype(bf16)
    sind = sind.astype(bf16)
    in_maps = []
    for c in range(N_CORES):
        g = c // 2
        wslc = np.concatenate([
            Wqkv[:, (2 * c) * HD:(2 * c) * HD + HD][:, ROPE_PERM],
            Wqkv[:, (2 * c + 1) * HD:(2 * c + 2) * HD][:, ROPE_PERM],
            Wqkv[:, 16 * HD + g * HD:16 * HD + (g + 1) * HD][:, ROPE_PERM],
            Wqkv[:, 20 * HD + g * HD:20 * HD + (g + 1) * HD],
        ], axis=1)
        wslc_p = np.ascontiguousarray(
            wslc.reshape(NDK, 128, 4 * HD).transpose(1, 0, 2)).astype(bf16)
        in_maps.append({
            "xT": xT,
            "wslc": wslc_p,
            "wo": np.ascontiguousarray(Wo[(2 * c) * HD:(2 * c + 2) * HD, :]).astype(bf16),
            "snk": np.ascontiguousarray(s[:, 2 * c:2 * c + 2]),
            "cosd": cosd,
            "sind": sind,
        })
    return in_maps


_CACHE = {}


def _get_exec():
    """Build the program once and return a cached jitted 8-core executor."""
    if "exec" in _CACHE:
        return _CACHE["exec"]

    import jax
    from jax.sharding import Mesh, PartitionSpec
    from jax.experimental.shard_map import shard_map
    from concourse.bass2jax import (_bass_exec_p, install_neuronx_cc_hook,
                                    partition_id_tensor)

    nc = _build_program()
    install_neuronx_cc_hook()

    partition_name = (nc.partition_id_tensor.name
                      if nc.partition_id_tensor else None)
    in_names, out_names, out_avals = [], [], []
    for alloc in nc.m.functions[0].allocations:
        if not isinstance(alloc, mybir.MemoryLocationSet):
            continue
        name = alloc.memorylocations[0].name
        if alloc.kind == "ExternalInput":
            if name != partition_name:
                in_names.append(name)
        elif alloc.kind == "ExternalOutput":
            out_names.append(name)
            out_avals.append(jax.core.ShapedArray(
                tuple(alloc.tensor_shape), mybir.dt.np(alloc.dtype)))
    n_params = len(in_names)
    all_names = in_names + out_names
    if partition_name is not None:
        all_names = all_names + [partition_name]

    def _body(*args):
        operands = list(args)
        if partition_name is not None:
            operands.append(partition_id_tensor())
        outs = _bass_exec_p.bind(
            *operands,
            out_avals=tuple(out_avals),
            in_names=tuple(all_names),
            out_names=tuple(out_names),
            lowering_input_output_aliases=(),
            sim_require_finite=True,
            sim_require_nnan=True,
            nc=nc,
        )
        return tuple(outs)

    devices = jax.devices()[:N_CORES]
    mesh = Mesh(np.asarray(devices), ("core",))
    n_outs = len(out_names)
    sharded = jax.jit(
        shard_map(_body, mesh=mesh,
                  in_specs=(PartitionSpec("core"),) * (n_params + n_outs),
                  out_specs=(PartitionSpec("core"),) * n_outs,
                  check_rep=False),
        keep_unused=True)

    state = {
        "sharded": sharded, "in_names": in_names, "out_names": out_names,
        "out_avals": out_avals, "mesh": mesh, "n_params": n_params,
        "nc": nc,
    }
    _CACHE["exec"] = state
    return state


def _run_cores(in_maps):
    ex = _get_exec()
    concat_in = [
        np.concatenate([np.asarray(m[name]) for m in in_maps], axis=0)
        for name in ex["in_names"]
    ]
    concat_zeros = [
        np.zeros((N_CORES * a.shape[0],) + tuple(a.shape[1:]), a.dtype)
        for a in ex["out_avals"]
    ]
    outs = ex["sharded"](*concat_in, *concat_zeros)
    name_to_i = {n: i for i, n in enumerate(ex["out_names"])}
    yi = name_to_i["y"]
    y_all = np.asarray(outs[yi]).astype(np.float32).reshape(N_CORES, L, D)
    return y_all


def kernel(x, Wqkv, Wo, s):
    in_maps = _make_in_maps(x, Wqkv, Wo, s)
    y_all = _run_cores(in_maps)
    out = y_all.sum(axis=0, dtype=np.float32)
    return out.reshape(1, L, D).astype(np.float32)


# revision 77
# speedup vs baseline: 1.4287x; 1.0178x over previous
"""Trainium2 Bass kernel for GroupedQueryAttention (inverted sliding-window mask + sink).

Full inputs in, full output out. Internally head-sharded across 8 NeuronCores:
core c handles q heads {2c, 2c+1} and kv head c//2, computes its partial
(x @ Wqkv_slice -> RoPE -> scores -> masked softmax w/ sink -> AV -> @ Wo_slice),
host sums the 8 partial outputs (the all-reduce).

v2: bf16 matmul inputs (host-cast), streamed x loads, post-exp multiplicative
masks on DVE, stream_shuffle RoPE, Wo tiles interleaved into next q-block.
"""

import os
import sys
from contextlib import ExitStack

sys.path.insert(0, "/opt/trn_rl_repo")

# jax must see the axon/neuron platform; a stray JAX_PLATFORMS=cpu would hide it.
if os.environ.get("JAX_PLATFORMS", "") == "cpu":
    os.environ["JAX_PLATFORMS"] = ""

import numpy as np

import concourse.bass as bass
import concourse.tile as tile
from concourse import bacc, mybir

F32 = mybir.dt.float32
BF16 = mybir.dt.bfloat16

LABELS = {}  # instruction name -> logical label (for trace analysis)


def _lbl(inst, label):
    try:
        LABELS[inst.ins.name] = label
    except Exception:
        pass
    return inst

N_CORES = 8
L = 2048
D = 2048
HD = 128
WINDOW = 1024
ROPE_BASE = 1024.0
SM_SCALE = 1.0 / float(np.sqrt(HD))

QB = 512          # q block (free dim of score tiles)
NQB = L // QB     # 4
NKT = L // HD     # 16 k tiles of 128
NDK = D // HD     # 16 contraction chunks for projections
NLB = L // QB     # 4 l-blocks for projection

# multiplicative-mask tiles are keyed by diff0 = q0 - k0 of the (k-tile, q-block)
MASK_DIFF0S = [0, -128, -256, -384, 640, 768, 896, 1024]
MASK_IDX = {d: i for i, d in enumerate(MASK_DIFF0S)}

# stream_shuffle permutes only within 32-partition quadrants, so q/k head
# dims are re-ordered (host side) to put each RoPE pair (d, d+64) 16 rows
# apart inside one quadrant: row 32s+i -> dim 16s+i, row 32s+16+i -> dim
# 64+16s+i (i<16). The shared permutation leaves q.k dot products unchanged.
SHUF16 = [(i + 16) % 32 for i in range(32)]
ROPE_PERM = np.array(
    [16 * s + i if i < 16 else 64 + 16 * s + (i - 16)
     for s in range(4) for i in range(32)])


def _classify(kt: int, qb: int):
    """masked band is 0 <= q-k <= WINDOW-1 (those entries get zeroed)."""
    d0 = QB * qb - HD * kt
    if 128 <= d0 <= 512:
        return "skip", None      # tile entirely inside the band -> contributes 0
    if d0 <= -512 or d0 >= 1152:
        return "full", None      # tile entirely outside the band -> no mask needed
    return "partial", MASK_IDX[d0]


def _build_program(dump=False):
    nc = bacc.Bacc("TRN2", target_bir_lowering=False, debug=False,
                   num_devices=N_CORES)
    dbg = {}
    if dump:
        for nm in ("dbg_q0", "dbg_q1", "dbg_k", "dbg_vT", "dbg_o0", "dbg_o1"):
            dbg[nm] = nc.dram_tensor(nm, [128, L], F32, kind="ExternalOutput").ap()
        dbg["dbg_v"] = nc.dram_tensor("dbg_v", [128, NKT * HD], F32,
                                      kind="ExternalOutput").ap()

    # host-packed to SBUF layout: [partition, chunk, col]
    xT_d = nc.dram_tensor("xT", [128, NDK, L], BF16, kind="ExternalInput").ap()
    wslc_d = nc.dram_tensor("wslc", [128, NDK, 4 * HD], BF16,
                            kind="ExternalInput").ap()
    wo_d = nc.dram_tensor("wo", [2 * HD, D], BF16, kind="ExternalInput").ap()
    snk_d = nc.dram_tensor("snk", [1, 2], F32, kind="ExternalInput").ap()
    cosd_d = nc.dram_tensor("cosd", [128, L], BF16, kind="ExternalInput").ap()
    sind_d = nc.dram_tensor("sind", [128, L], BF16, kind="ExternalInput").ap()
    # partial y ships bf16 (halves the dominant write traffic); host sums f32
    y_d = nc.dram_tensor("y", [L, D], BF16, kind="ExternalOutput").ap()

    with tile.TileContext(nc) as tc, ExitStack() as stk:
        persist = stk.enter_context(tc.tile_pool(name="persist", bufs=1))

        # ---- persistent SBUF tensors ----
        wslc_sb = persist.tile([128, NDK, 4 * HD], BF16, tag="wslc")
        wo_sb = persist.tile([128, 2, D], BF16, tag="wo")
        qT = [persist.tile([128, L], BF16, tag=f"qT{h}", name=f"qT{h}") for h in range(2)]
        kT = persist.tile([128, L], BF16, tag="kT")
        vT = persist.tile([128, L], BF16, tag="vT")
        v_sb = persist.tile([128, NKT, HD], BF16, tag="v")
        # one tile per (head, q-block): slices of a single tensor would
        # false-couple interleaved Wo reads to later rb-mul writes
        oT = [[persist.tile([128, QB], BF16, tag=f"oT{h}_{b}", name=f"oT{h}_{b}")
               for b in range(NQB)] for h in range(2)]
        cosd_sb = persist.tile([128, L], BF16, tag="cosd")
        sind_sb = persist.tile([128, L], BF16, tag="sind")
        masks = persist.tile([128, len(MASK_DIFF0S), QB], BF16, tag="masks")
        ident = persist.tile([128, 128], BF16, tag="ident")
        ones_bf = persist.tile([128, 1], BF16, tag="ones")
        snk_sb = persist.tile([1, 2], F32, tag="snk")
        exps_sb = persist.tile([1, 2], F32, tag="exps")

        # ---- weight loads ----
        # wslc split across SP and Act queues in small pieces so the
        # transfers interleave with the x-chunk stream on the DMA engines
        # (per-DMA queue hold is ~1.3us + transfer, so one queue can't feed
        # a chunk every 0.85us alone)
        for eng, pieces in ((nc.sync, ((0, 1), (2, 4), (6, 8), (10, 12), (14, 16))),
                            (nc.scalar, ((1, 2), (4, 6), (8, 10), (12, 14)))):
            for a, b in pieces:
                eng.dma_start(wslc_sb[:, a:b, :], wslc_d[:, a:b, :])
        nc.scalar.dma_start(snk_sb[:], snk_d[:])

        nc.vector.memset(ones_bf[:], 1.0)
        # exp of the two sink logits
        nc.scalar.activation(exps_sb[:], snk_sb[:], mybir.ActivationFunctionType.Exp)

        def build_masks():
            # multiplicative masks: 0 where 0 <= (q-k) <= WINDOW-1, else 1
            # (on Pool, emitted mid-phase-A: needed only at phase B)
            for i, d0 in enumerate(MASK_DIFF0S):
                m = masks[:, i, :]
                nc.gpsimd.memset(m, 1.0)
                if d0 <= 0:
                    # keep 1 where q-k < 0, i.e. kp - qf - d0 - 1 >= 0
                    nc.gpsimd.affine_select(
                        out=m, in_=m, compare_op=mybir.AluOpType.is_ge,
                        fill=0.0, base=-d0 - 1, channel_multiplier=1,
                        pattern=[[-1, QB]])
                else:
                    # keep 1 where q-k >= WINDOW, i.e. qf - kp + d0 - WINDOW >= 0
                    nc.gpsimd.affine_select(
                        out=m, in_=m, compare_op=mybir.AluOpType.is_ge,
                        fill=0.0, base=d0 - WINDOW, channel_multiplier=-1,
                        pattern=[[1, QB]])

        # ================= Phase A: QKV projection (transposed) =================
        # pT[c*128+r, l] = sum_d wslc[d, c*128+r] * x[l, d];  cols c = q0,q1,k,v
        col_dst = [qT[0], qT[1], kT, vT]

        xt_sb = persist.tile([128, NDK, L], BF16, tag="xt")

        # rope pool spans phases A and B: the last l-block's RoPE and
        # transposes are deferred into early attention
        rp = stk.enter_context(tc.tile_pool(name="rope", bufs=4))

        def rope_slice(t, lb):
            ls = slice(lb * QB, (lb + 1) * QB)
            partner = rp.tile([128, QB], BF16, tag="partner")
            nc.vector.stream_shuffle(partner[:], t[:, ls], SHUF16)
            tmp = rp.tile([128, QB], BF16, tag="ropetmp")
            nc.vector.tensor_mul(tmp[:], t[:, ls], cosd_sb[:, ls])
            nc.vector.tensor_mul(partner[:], partner[:], sind_sb[:, ls])
            nc.vector.tensor_add(t[:, ls], tmp[:], partner[:])

        def v_transposes(lb, pool, tag="vt"):
            pt = pool.tile([128, 4, HD], BF16, tag=tag)
            for j in range(4):
                t = 4 * lb + j
                _lbl(nc.tensor.transpose(
                    pt[:, j, :], vT[:, t * 128:(t + 1) * 128], ident[:]),
                    f"transp{t}")
            # DVE, not Act: Act must stay clear to start the exps promptly
            # (Pool can't read PSUM)
            nc.vector.tensor_copy(v_sb[:, 4 * lb:4 * lb + 4, :], pt[:])

        with tc.tile_pool(name="psA", bufs=6, space="PSUM") as psA, \
             tc.tile_pool(name="psT", bufs=1, space="PSUM") as psT:

            prev_v_lb = None
            for lb in range(NLB):
                ls = slice(lb * QB, (lb + 1) * QB)
                # x loads ride Pool's software DGE: the transfer is async, so
                # the Pool engine is only held ~1us per DMA. First block goes
                # per-chunk so PE starts as soon as possible.
                if lb == 0:
                    # Pool SWDGE issues ~1us apart vs PE consuming a chunk
                    # every ~0.85us: single chunks first, then pairs
                    for a, b in ((0, 1), (1, 2), (2, 4), (4, 6), (6, 8),
                                 (8, 10), (10, 13), (13, 16)):
                        nc.gpsimd.dma_start(
                            xt_sb[:, a:b, ls], xT_d[:, a:b, ls])
                else:
                    for g in range(4):
                        nc.gpsimd.dma_start(
                            xt_sb[:, 4 * g:4 * g + 4, ls],
                            xT_d[:, 4 * g:4 * g + 4, ls])
                # cos/sin slices arrive just before this block's RoPE; wo is
                # not needed until phase C — keep them off the early window
                nc.scalar.dma_start(cosd_sb[:, ls], cosd_d[:, ls])
                nc.scalar.dma_start(sind_sb[:, ls], sind_d[:, ls])
                if lb == 1:
                    for h in range(2):
                        nc.scalar.dma_start(wo_sb[:, h, :],
                                            wo_d[h * 128:(h + 1) * 128, :])
                if lb == 0:
                    # identity for PE transposes (needed from lb1 on)
                    nc.gpsimd.memset(ident[:], 0.0)
                    nc.gpsimd.affine_select(
                        out=ident[:], in_=ident[:],
                        compare_op=mybir.AluOpType.not_equal,
                        fill=1.0, base=0, channel_multiplier=1,
                        pattern=[[-1, 128]])
                if lb == NLB - 1:
                    build_masks()
                if lb < NLB - 1:
                    # chunk-major: all 4 column psums fill together while the
                    # x chunks stream in
                    psums = [psA.tile([128, QB], F32, tag="proj",
                                      name=f"psproj{c}") for c in range(4)]
                    for k in range(NDK):
                        for c in range(4):
                            _lbl(nc.tensor.matmul(
                                psums[c][:],
                                wslc_sb[:, k, c * 128:(c + 1) * 128],
                                xt_sb[:, k, ls],
                                start=(k == 0), stop=(k == NDK - 1)),
                                f"proj_lb{lb}_k{k}_c{c}")
                    if prev_v_lb is not None:
                        v_transposes(prev_v_lb, psT)
                    # copies psum -> bf16 SBUF; k,q0 on Act; q1,v on DVE
                    nc.scalar.copy(kT[:, ls], psums[2][:])
                    nc.scalar.copy(qT[0][:, ls], psums[0][:])
                    nc.vector.tensor_copy(qT[1][:, ls], psums[1][:])
                    nc.vector.tensor_copy(vT[:, ls], psums[3][:])
                    rope_slice(kT, lb)
                    rope_slice(qT[0], lb)
                    rope_slice(qT[1], lb)
                else:
                    # last block goes column-major (its x data is resident
                    # already): each column's copy + RoPE overlaps the next
                    # column's matmuls, so the attention pool-open barrier
                    # only waits on the final v copy
                    for c, dst, eng, rope in ((2, kT, nc.scalar, True),
                                              (0, qT[0], nc.scalar, True),
                                              (1, qT[1], nc.vector, True),
                                              (3, vT, nc.vector, False)):
                        ps = psA.tile([128, QB], F32, tag="proj",
                                      name=f"pscol{c}")
                        for k in range(NDK):
                            _lbl(nc.tensor.matmul(
                                ps[:],
                                wslc_sb[:, k, c * 128:(c + 1) * 128],
                                xt_sb[:, k, ls],
                                start=(k == 0), stop=(k == NDK - 1)),
                                f"proj_lb{lb}_k{k}_c{c}")
                        if c == 2 and prev_v_lb is not None:
                            v_transposes(prev_v_lb, psT)
                        if eng is nc.scalar:
                            nc.scalar.copy(dst[:, ls], ps[:])
                        else:
                            nc.vector.tensor_copy(dst[:, ls], ps[:])
                        if rope:
                            rope_slice(dst, lb)
                prev_v_lb = lb

        # ============ Phase B+C: attention + output projection ============
        ycnt = [0]
        ystage = {}

        def make_emit_y(psY, sbY, act_share):
            def emit_y_tile(qt, nb, act_ok=True):
                # one [128,512] y tile: 2 Wo matmuls + copy into a per-row
                # staging buffer; the whole [128,2048] row block ships as one
                # DMA (SP holds its SEQ for the full transfer; Pool's SWDGE
                # is async, so alternate).
                qts = slice(qt * 128, (qt + 1) * 128)
                ns = slice(nb * QB, (nb + 1) * QB)
                if nb == 0:
                    ystage[qt] = sbY.tile([128, D // QB, QB], BF16, tag="ysb",
                                          name=f"ystage{qt}")
                psum_y = psY.tile([128, QB], F32, tag="y")
                for h in range(2):
                    _lbl(nc.tensor.matmul(
                        psum_y[:],
                        oT[h][qt // 4][:, (qt % 4) * 128:(qt % 4 + 1) * 128],
                        wo_sb[:, h, ns],
                        start=(h == 0), stop=(h == 1)),
                        f"y_qt{qt}_nb{nb}_h{h}")
                # copy engine: Act where the current group's exp load is
                # light (act_ok), else mostly DVE
                if ycnt[0] % (2 if (act_share or act_ok) else 4) == 1:
                    nc.scalar.copy(ystage[qt][:, nb, :], psum_y[:])
                else:
                    nc.vector.tensor_copy(ystage[qt][:, nb, :], psum_y[:])
                ycnt[0] += 1
                # Pool SWDGE (async) for in-group rows; final-drain rows are
                # split across SP/Act/Pool queues piece-by-piece so the last
                # flush is a single small racing transfer per queue
                # ship each half as soon as its copies land: SP takes the
                # first (its HWDGE gen is fast and the queue is idle), Pool's
                # single SWDGE gen per row keeps the final flush short
                if nb == 1:
                    nc.sync.dma_start(y_d[qts, 0:D // 2], ystage[qt][:, 0:2, :])
                elif nb == 3:
                    nc.gpsimd.dma_start(y_d[qts, D // 2:D],
                                        ystage[qt][:, 2:4, :])
                    del ystage[qt]
            return emit_y_tile

        with tc.tile_pool(name="psS", bufs=4, space="PSUM") as psS, \
             tc.tile_pool(name="psO", bufs=2, space="PSUM") as psO, \
             tc.tile_pool(name="psD", bufs=1, space="PSUM") as psD, \
             tc.tile_pool(name="psY", bufs=1, space="PSUM") as psY, \
             tc.tile_pool(name="epool", bufs=8) as epool, \
             tc.tile_pool(name="sbB", bufs=4) as sbB, \
             tc.tile_pool(name="sbY", bufs=4) as sbY:

            emit_y_tile = make_emit_y(psY, sbY, act_share=False)

            pending_y = []
            LAG = 3  # tiles between score emission and its den/AV, hiding exp
            for qb in range(NQB):
                qs = slice(qb * QB, (qb + 1) * QB)
                for h in range(2):
                    acts = [(kt, _classify(kt, qb)) for kt in range(NKT)]
                    acts = [(kt, c, mi) for kt, (c, mi) in acts if c != "skip"]
                    # full tiles first: partial tiles' den/AV depend on DVE
                    # mask-muls, so give DVE the whole group to produce them
                    acts = ([a for a in acts if a[1] == "full"]
                            + [a for a in acts if a[1] == "partial"])
                    n_act = len(acts)
                    first_group = (qb == 0 and h == 0)
                    psum_o = psO.tile([128, QB], F32, tag="o")
                    psum_den = psD.tile([1, QB], F32, tag="den")
                    e_use = [None] * n_act
                    for i in range(n_act + LAG):
                        if i < n_act:
                            kt, cls, mi = acts[i]
                            psum_s = psS.tile([128, QB], F32, tag="s")
                            _lbl(nc.tensor.matmul(
                                psum_s[:],
                                kT[:, kt * 128:(kt + 1) * 128],
                                qT[h][:, qs],
                                start=True, stop=True),
                                f"score_h{h}_qb{qb}_kt{kt}")
                            e_sb = epool.tile([128, QB], BF16, tag="e")
                            nc.scalar.activation(
                                e_sb[:], psum_s[:],
                                mybir.ActivationFunctionType.Exp,
                                scale=SM_SCALE)
                            if cls == "partial":
                                e_m = epool.tile([128, QB], BF16, tag="em")
                                nc.vector.tensor_mul(
                                    e_m[:], e_sb[:], masks[:, mi, :])
                                e_use[i] = e_m
                            else:
                                e_use[i] = e_sb
                        if first_group and i == 4:
                            # deferred last-block v transposes, borrowing a
                            # score-pool buffer (same bank size)
                            v_transposes(NLB - 1, psS, tag="s")
                        # interleave one deferred y tile of the previous q
                        # block BEFORE den/AV: fills PE while the exp chain
                        # completes and issues the staging copy earlier
                        if pending_y and i >= 2 and not (qb == NQB - 1
                                                         and h == 1):
                            # late q-blocks run 12-tile groups: Act has slack.
                            # nothing pops in the very last group — leftovers
                            # drain through the deep-buffered pools instead
                            emit_y_tile(*pending_y.pop(0), act_ok=(qb >= 2))
                        j = i - LAG
                        if 0 <= j < n_act:
                            ktj = acts[j][0]
                            _lbl(nc.tensor.matmul(
                                psum_den[:], ones_bf[:],
                                e_use[j][:],
                                start=(j == 0), stop=(j == n_act - 1)),
                                f"den_h{h}_qb{qb}_kt{ktj}")
                            _lbl(nc.tensor.matmul(
                                psum_o[:], v_sb[:, ktj, :],
                                e_use[j][:],
                                start=(j == 0), stop=(j == n_act - 1)),
                                f"av_h{h}_qb{qb}_kt{ktj}")
                            e_use[j] = None
                    den_sb = sbB.tile([1, QB], F32, tag="densb")
                    nc.scalar.activation(
                        den_sb[:], psum_den[:],
                        mybir.ActivationFunctionType.Identity,
                        bias=exps_sb[0:1, h:h + 1])
                    r_sb = sbB.tile([1, QB], F32, tag="rsb")
                    nc.vector.reciprocal(r_sb[:], den_sb[:])
                    rb = sbB.tile([128, QB], F32, tag="rb")
                    nc.gpsimd.partition_broadcast(rb[:], r_sb[:])
                    nc.vector.tensor_mul(oT[h][qb][:], psum_o[:], rb[:])
                pending_y.extend(
                    (qb * (QB // 128) + j, nb)
                    for j in range(QB // 128) for nb in range(D // QB))

        # final-qb y drain: attention pools are done, so rebuild with deep
        # buffering and let the copies use both Act and DVE
        with tc.tile_pool(name="psY2", bufs=4, space="PSUM") as psY2, \
             tc.tile_pool(name="sbY2", bufs=4) as sbY2:
            emit_y_tile = make_emit_y(psY2, sbY2, act_share=True)
            while pending_y:
                emit_y_tile(*pending_y.pop(0))

        if dump:
            with tc.tile_pool(name="dbgp", bufs=2) as dbgp:
                for nm, t in (("dbg_q0", qT[0]), ("dbg_q1", qT[1]),
                              ("dbg_k", kT), ("dbg_vT", vT)):
                    f = dbgp.tile([128, L], F32, tag="dbgf", name=f"f{nm}")
                    nc.scalar.copy(f[:], t[:])
                    nc.sync.dma_start(dbg[nm], f[:])
                for h in range(2):
                    f = dbgp.tile([128, L], F32, tag="dbgf", name=f"fo{h}")
                    for b in range(NQB):
                        nc.scalar.copy(f[:, b * QB:(b + 1) * QB], oT[h][b][:])
                    nc.sync.dma_start(dbg[f"dbg_o{h}"], f[:])
                fv = dbgp.tile([128, NKT, HD], F32, tag="dbgf", name="fv")
                nc.scalar.copy(fv[:], v_sb[:])
                nc.sync.dma_start(dbg["dbg_v"], fv[:])

    nc.compile()
    return nc


def _rope_tables():
    """cos/sin tables in the permuted row order (see ROPE_PERM)."""
    freqs = (1.0 / ROPE_BASE) ** np.linspace(0.0, 1.0, num=HD // 4,
                                             dtype=np.float32)
    theta = freqs[:, None].astype(np.float32) * np.arange(L, dtype=np.float32)[None, :]
    cos32 = np.cos(theta).astype(np.float32)   # (32, L), freq j
    sin32 = np.sin(theta).astype(np.float32)
    cosd = np.ones((128, L), dtype=np.float32)
    sind = np.zeros((128, L), dtype=np.float32)
    for sq in range(2):   # quadrants 0,1 carry the 32 active freqs
        fr = slice(16 * sq, 16 * sq + 16)
        cosd[32 * sq:32 * sq + 16] = cos32[fr]
        cosd[32 * sq + 16:32 * sq + 32] = cos32[fr]
        sind[32 * sq:32 * sq + 16] = sin32[fr]
        sind[32 * sq + 16:32 * sq + 32] = -sin32[fr]
    return cosd, sind


def _make_in_maps(x, Wqkv, Wo, s):
    bf16 = mybir.dt.np(BF16)
    x = np.asarray(x, dtype=np.float32)
    Wqkv = np.asarray(Wqkv, dtype=np.float32)
    Wo = np.asarray(Wo, dtype=np.float32)
    s = np.asarray(s, dtype=np.float32)
    # pack to SBUF layout [partition, chunk, col]: xT[p, k, l] = x[l, k*128+p]
    xT = np.ascontiguousarray(
        x.reshape(L, NDK, 128).transpose(2, 1, 0)).astype(bf16)
    cosd, sind = _rope_tables()
    cosd = cosd.astype(bf16)
    sind = sind.astype(bf16)
    in_maps = []
    for c in range(N_CORES):
        g = c // 2
        wslc = np.concatenate([
            Wqkv[:, (2 * c) * HD:(2 * c) * HD + HD][:, ROPE_PERM],
            Wqkv[:, (2 * c + 1) * HD:(2 * c + 2) * HD][:, ROPE_PERM],
            Wqkv[:, 16 * HD + g * HD:16 * HD + (g + 1) * HD][:, ROPE_PERM],
            Wqkv[:, 20 * HD + g * HD:20 * HD + (g + 1) * HD],
        ], axis=1)
        wslc_p = np.ascontiguousarray(
            wslc.reshape(NDK, 128, 4 * HD).transpose(1, 0, 2)).astype(bf16)
        in_maps.append({
            "xT": xT,
            "wslc": wslc_p,
            "wo": np.ascontiguousarray(Wo[(2 * c) * HD:(2 * c + 2) * HD, :]).astype(bf16),
            "snk": np.ascontiguousarray(s[:, 2 * c:2 * c + 2]),
            "cosd": cosd,
            "sind": sind,
        })
    return in_maps


_CACHE = {}


def _get_exec():
    """Build the program once and return a cached jitted 8-core executor."""
    if "exec" in _CACHE:
        return _CACHE["exec"]

    import jax
    from jax.sharding import Mesh, PartitionSpec
    from jax.experimental.shard_map import shard_map
    from concourse.bass2jax import (_bass_exec_p, install_neuronx_cc_hook,
                                    partition_id_tensor)

    nc = _build_program()
    install_neuronx_cc_hook()

    partition_name = (nc.partition_id_tensor.name
                      if nc.partition_id_tensor else None)
    in_names, out_names, out_avals = [], [], []
    for alloc in nc.m.functions[0].allocations:
        if not isinstance(alloc, mybir.MemoryLocationSet):
            continue
        name = alloc.memorylocations[0].name
        if alloc.kind == "ExternalInput":
            if name != partition_name:
                in_names.append(name)
        elif alloc.kind == "ExternalOutput":
            out_names.append(name)
            out_avals.append(jax.core.ShapedArray(
                tuple(alloc.tensor_shape), mybir.dt.np(alloc.dtype)))
    n_params = len(in_names)
    all_names = in_names + out_names
    if partition_name is not None:
        all_names = all_names + [partition_name]

    def _body(*args):
        operands = list(args)
        if partition_name is not None:
            operands.append(partition_id_tensor())
        outs = _bass_exec_p.bind(
            *operands,
            out_avals=tuple(out_avals),
            in_names=tuple(all_names),
            out_names=tuple(out_names),
            lowering_input_output_aliases=(),
            sim_require_finite=True,
            sim_require_nnan=True,
            nc=nc,
        )
        return tuple(outs)

    devices = jax.devices()[:N_CORES]
    mesh = Mesh(np.asarray(devices), ("core",))
    n_outs = len(out_names)
    sharded = jax.jit(
        shard_map(_body, mesh=mesh,
                  in_specs=(PartitionSpec("core"),) * (n_params + n_outs),
                  out_specs=(PartitionSpec("core"),) * n_outs,
                  check_rep=False),
        keep_unused=True)

    state = {
        "sharded": sharded, "in_names": in_names, "out_names": out_names,
        "out_avals": out_avals, "mesh": mesh, "n_params": n_params,
        "nc": nc,
    }
    _CACHE["exec"] = state
    return state


def _run_cores(in_maps):
    ex = _get_exec()
    concat_in = [
        np.concatenate([np.asarray(m[name]) for m in in_maps], axis=0)
        for name in ex["in_names"]
    ]
    concat_zeros = [
        np.zeros((N_CORES * a.shape[0],) + tuple(a.shape[1:]), a.dtype)
        for a in ex["out_avals"]
    ]
    outs = ex["sharded"](*concat_in, *concat_zeros)
    name_to_i = {n: i for i, n in enumerate(ex["out_names"])}
    yi = name_to_i["y"]
    y_all = np.asarray(outs[yi]).astype(np.float32).reshape(N_CORES, L, D)
    return y_all


def kernel(x, Wqkv, Wo, s):
    in_maps = _make_in_maps(x, Wqkv, Wo, s)
    y_all = _run_cores(in_maps)
    out = y_all.sum(axis=0, dtype=np.float32)
    return out.reshape(1, L, D).astype(np.float32)


# revision 84
# speedup vs baseline: 1.4395x; 1.0076x over previous
"""Trainium2 Bass kernel for GroupedQueryAttention (inverted sliding-window mask + sink).

Full inputs in, full output out. Internally head-sharded across 8 NeuronCores:
core c handles q heads {2c, 2c+1} and kv head c//2, computes its partial
(x @ Wqkv_slice -> RoPE -> scores -> masked softmax w/ sink -> AV -> @ Wo_slice),
host sums the 8 partial outputs (the all-reduce).

v2: bf16 matmul inputs (host-cast), streamed x loads, post-exp multiplicative
masks on DVE, stream_shuffle RoPE, Wo tiles interleaved into next q-block.
"""

import os
import sys
from contextlib import ExitStack

sys.path.insert(0, "/opt/trn_rl_repo")

# jax must see the axon/neuron platform; a stray JAX_PLATFORMS=cpu would hide it.
if os.environ.get("JAX_PLATFORMS", "") == "cpu":
    os.environ["JAX_PLATFORMS"] = ""

import numpy as np

import concourse.bass as bass
import concourse.tile as tile
from concourse import bacc, mybir

F32 = mybir.dt.float32
BF16 = mybir.dt.bfloat16

LABELS = {}  # instruction name -> logical label (for trace analysis)


def _lbl(inst, label):
    try:
        LABELS[inst.ins.name] = label
    except Exception:
        pass
    return inst

N_CORES = 8
L = 2048
D = 2048
HD = 128
WINDOW = 1024
ROPE_BASE = 1024.0
SM_SCALE = 1.0 / float(np.sqrt(HD))

QB = 512          # q block (free dim of score tiles)
NQB = L // QB     # 4
NKT = L // HD     # 16 k tiles of 128
NDK = D // HD     # 16 contraction chunks for projections
NLB = L // QB     # 4 l-blocks for projection

# multiplicative-mask tiles are keyed by diff0 = q0 - k0 of the (k-tile, q-block)
MASK_DIFF0S = [0, -128, -256, -384, 640, 768, 896, 1024]
MASK_IDX = {d: i for i, d in enumerate(MASK_DIFF0S)}

# stream_shuffle permutes only within 32-partition quadrants, so q/k head
# dims are re-ordered (host side) to put each RoPE pair (d, d+64) 16 rows
# apart inside one quadrant: row 32s+i -> dim 16s+i, row 32s+16+i -> dim
# 64+16s+i (i<16). The shared permutation leaves q.k dot products unchanged.
SHUF16 = [(i + 16) % 32 for i in range(32)]
ROPE_PERM = np.array(
    [16 * s + i if i < 16 else 64 + 16 * s + (i - 16)
     for s in range(4) for i in range(32)])


def _classify(kt: int, qb: int):
    """masked band is 0 <= q-k <= WINDOW-1 (those entries get zeroed)."""
    d0 = QB * qb - HD * kt
    if 128 <= d0 <= 512:
        return "skip", None      # tile entirely inside the band -> contributes 0
    if d0 <= -512 or d0 >= 1152:
        return "full", None      # tile entirely outside the band -> no mask needed
    return "partial", MASK_IDX[d0]


def _build_program(dump=False):
    nc = bacc.Bacc("TRN2", target_bir_lowering=False, debug=False,
                   num_devices=N_CORES)
    dbg = {}
    if dump:
        for nm in ("dbg_q0", "dbg_q1", "dbg_k", "dbg_vT", "dbg_o0", "dbg_o1"):
            dbg[nm] = nc.dram_tensor(nm, [128, L], F32, kind="ExternalOutput").ap()
        dbg["dbg_v"] = nc.dram_tensor("dbg_v", [128, NKT * HD], F32,
                                      kind="ExternalOutput").ap()

    # host-packed to SBUF layout: [partition, chunk, col]
    xT_d = nc.dram_tensor("xT", [128, NDK, L], BF16, kind="ExternalInput").ap()
    wslc_d = nc.dram_tensor("wslc", [128, NDK, 4 * HD], BF16,
                            kind="ExternalInput").ap()
    wo_d = nc.dram_tensor("wo", [2 * HD, D], BF16, kind="ExternalInput").ap()
    snk_d = nc.dram_tensor("snk", [1, 2], F32, kind="ExternalInput").ap()
    cosd_d = nc.dram_tensor("cosd", [128, L], BF16, kind="ExternalInput").ap()
    sind_d = nc.dram_tensor("sind", [128, L], BF16, kind="ExternalInput").ap()
    # partial y ships bf16 (halves the dominant write traffic); host sums f32
    y_d = nc.dram_tensor("y", [L, D], BF16, kind="ExternalOutput").ap()

    with tile.TileContext(nc) as tc, ExitStack() as stk:
        persist = stk.enter_context(tc.tile_pool(name="persist", bufs=1))

        # ---- persistent SBUF tensors ----
        wslc_sb = persist.tile([128, NDK, 4 * HD], BF16, tag="wslc")
        wo_sb = persist.tile([128, 2, D], BF16, tag="wo")
        qT = [persist.tile([128, L], BF16, tag=f"qT{h}", name=f"qT{h}") for h in range(2)]
        kT = persist.tile([128, L], BF16, tag="kT")
        vT = persist.tile([128, L], BF16, tag="vT")
        v_sb = persist.tile([128, NKT, HD], BF16, tag="v")
        # one tile per (head, q-block): slices of a single tensor would
        # false-couple interleaved Wo reads to later rb-mul writes
        oT = [[persist.tile([128, QB], BF16, tag=f"oT{h}_{b}", name=f"oT{h}_{b}")
               for b in range(NQB)] for h in range(2)]
        cosd_sb = persist.tile([128, L], BF16, tag="cosd")
        sind_sb = persist.tile([128, L], BF16, tag="sind")
        masks = persist.tile([128, len(MASK_DIFF0S), QB], BF16, tag="masks")
        ident = persist.tile([128, 128], BF16, tag="ident")
        ones_bf = persist.tile([128, 1], BF16, tag="ones")
        snk_sb = persist.tile([1, 2], F32, tag="snk")
        exps_sb = persist.tile([1, 2], F32, tag="exps")

        # ---- weight loads ----
        # wslc split across SP and Act queues in small pieces so the
        # transfers interleave with the x-chunk stream on the DMA engines
        # (per-DMA queue hold is ~1.3us + transfer, so one queue can't feed
        # a chunk every 0.85us alone)
        for eng, pieces in ((nc.sync, ((0, 1), (2, 4), (6, 8), (8, 10),
                                       (10, 12), (12, 14), (14, 16))),
                            (nc.scalar, ((1, 2), (4, 6)))):
            for a, b in pieces:
                eng.dma_start(wslc_sb[:, a:b, :], wslc_d[:, a:b, :])
        nc.scalar.dma_start(snk_sb[:], snk_d[:])

        nc.vector.memset(ones_bf[:], 1.0)
        # exp of the two sink logits
        nc.scalar.activation(exps_sb[:], snk_sb[:], mybir.ActivationFunctionType.Exp)

        def build_masks():
            # multiplicative masks: 0 where 0 <= (q-k) <= WINDOW-1, else 1
            # (on Pool, emitted mid-phase-A: needed only at phase B)
            for i, d0 in enumerate(MASK_DIFF0S):
                m = masks[:, i, :]
                nc.gpsimd.memset(m, 1.0)
                if d0 <= 0:
                    # keep 1 where q-k < 0, i.e. kp - qf - d0 - 1 >= 0
                    nc.gpsimd.affine_select(
                        out=m, in_=m, compare_op=mybir.AluOpType.is_ge,
                        fill=0.0, base=-d0 - 1, channel_multiplier=1,
                        pattern=[[-1, QB]])
                else:
                    # keep 1 where q-k >= WINDOW, i.e. qf - kp + d0 - WINDOW >= 0
                    nc.gpsimd.affine_select(
                        out=m, in_=m, compare_op=mybir.AluOpType.is_ge,
                        fill=0.0, base=d0 - WINDOW, channel_multiplier=-1,
                        pattern=[[1, QB]])

        # ================= Phase A: QKV projection (transposed) =================
        # pT[c*128+r, l] = sum_d wslc[d, c*128+r] * x[l, d];  cols c = q0,q1,k,v
        col_dst = [qT[0], qT[1], kT, vT]

        xt_sb = persist.tile([128, NDK, L], BF16, tag="xt")

        # rope pool spans phases A and B: the last l-block's RoPE and
        # transposes are deferred into early attention
        rp = stk.enter_context(tc.tile_pool(name="rope", bufs=4))

        def rope_slice(t, lb):
            ls = slice(lb * QB, (lb + 1) * QB)
            partner = rp.tile([128, QB], BF16, tag="partner")
            nc.vector.stream_shuffle(partner[:], t[:, ls], SHUF16)
            tmp = rp.tile([128, QB], BF16, tag="ropetmp")
            nc.vector.tensor_mul(tmp[:], t[:, ls], cosd_sb[:, ls])
            nc.vector.tensor_mul(partner[:], partner[:], sind_sb[:, ls])
            nc.vector.tensor_add(t[:, ls], tmp[:], partner[:])

        def v_transposes(lb, pool, tag="vt"):
            pt = pool.tile([128, 4, HD], BF16, tag=tag)
            for j in range(4):
                t = 4 * lb + j
                _lbl(nc.tensor.transpose(
                    pt[:, j, :], vT[:, t * 128:(t + 1) * 128], ident[:]),
                    f"transp{t}")
            # DVE, not Act: Act must stay clear to start the exps promptly
            # (Pool can't read PSUM)
            nc.vector.tensor_copy(v_sb[:, 4 * lb:4 * lb + 4, :], pt[:])

        with tc.tile_pool(name="psA", bufs=6, space="PSUM") as psA, \
             tc.tile_pool(name="psT", bufs=1, space="PSUM") as psT:

            prev_v_lb = None
            for lb in range(NLB):
                ls = slice(lb * QB, (lb + 1) * QB)
                # x loads ride Pool's software DGE: the transfer is async, so
                # the Pool engine is only held ~1us per DMA. First block goes
                # per-chunk so PE starts as soon as possible.
                if lb == 0:
                    # Pool SWDGE issues ~1us apart vs PE consuming a chunk
                    # every ~0.85us: single chunks first, then pairs
                    for a, b in ((0, 2), (2, 4), (4, 6), (6, 8),
                                 (8, 10), (10, 13), (13, 16)):
                        nc.gpsimd.dma_start(
                            xt_sb[:, a:b, ls], xT_d[:, a:b, ls])
                else:
                    for g in range(4):
                        nc.gpsimd.dma_start(
                            xt_sb[:, 4 * g:4 * g + 4, ls],
                            xT_d[:, 4 * g:4 * g + 4, ls])
                # cos/sin slices arrive just before this block's RoPE; wo is
                # not needed until phase C — keep them off the early window
                nc.scalar.dma_start(cosd_sb[:, ls], cosd_d[:, ls])
                nc.scalar.dma_start(sind_sb[:, ls], sind_d[:, ls])
                if lb == 1:
                    for h in range(2):
                        nc.scalar.dma_start(wo_sb[:, h, :],
                                            wo_d[h * 128:(h + 1) * 128, :])
                if lb == 0:
                    # identity for PE transposes (needed from lb1 on)
                    nc.gpsimd.memset(ident[:], 0.0)
                    nc.gpsimd.affine_select(
                        out=ident[:], in_=ident[:],
                        compare_op=mybir.AluOpType.not_equal,
                        fill=1.0, base=0, channel_multiplier=1,
                        pattern=[[-1, 128]])
                if lb == NLB - 1:
                    build_masks()
                if lb < NLB - 1:
                    # chunk-major: all 4 column psums fill together while the
                    # x chunks stream in
                    psums = [psA.tile([128, QB], F32, tag="proj",
                                      name=f"psproj{c}") for c in range(4)]
                    for k in range(NDK):
                        for c in range(4):
                            _lbl(nc.tensor.matmul(
                                psums[c][:],
                                wslc_sb[:, k, c * 128:(c + 1) * 128],
                                xt_sb[:, k, ls],
                                start=(k == 0), stop=(k == NDK - 1)),
                                f"proj_lb{lb}_k{k}_c{c}")
                    if prev_v_lb is not None:
                        v_transposes(prev_v_lb, psT)
                    # copies psum -> bf16 SBUF; k,q0 on Act; q1,v on DVE
                    nc.scalar.copy(kT[:, ls], psums[2][:])
                    nc.scalar.copy(qT[0][:, ls], psums[0][:])
                    nc.vector.tensor_copy(qT[1][:, ls], psums[1][:])
                    nc.vector.tensor_copy(vT[:, ls], psums[3][:])
                    rope_slice(kT, lb)
                    rope_slice(qT[0], lb)
                    rope_slice(qT[1], lb)
                else:
                    # last block goes column-major (its x data is resident
                    # already): each column's copy + RoPE overlaps the next
                    # column's matmuls, so the attention pool-open barrier
                    # only waits on the final v copy
                    for c, dst, eng, rope in ((2, kT, nc.scalar, True),
                                              (0, qT[0], nc.scalar, True),
                                              (1, qT[1], nc.vector, True),
                                              (3, vT, nc.vector, False)):
                        ps = psA.tile([128, QB], F32, tag="proj",
                                      name=f"pscol{c}")
                        for k in range(NDK):
                            _lbl(nc.tensor.matmul(
                                ps[:],
                                wslc_sb[:, k, c * 128:(c + 1) * 128],
                                xt_sb[:, k, ls],
                                start=(k == 0), stop=(k == NDK - 1)),
                                f"proj_lb{lb}_k{k}_c{c}")
                        if c == 2 and prev_v_lb is not None:
                            v_transposes(prev_v_lb, psT)
                        if eng is nc.scalar:
                            nc.scalar.copy(dst[:, ls], ps[:])
                        else:
                            nc.vector.tensor_copy(dst[:, ls], ps[:])
                        if rope:
                            rope_slice(dst, lb)
                prev_v_lb = lb

        # ============ Phase B+C: attention + output projection ============
        ycnt = [0]
        ystage = {}

        def make_emit_y(psY, sbY, act_share):
            def emit_y_tile(qt, nb, act_ok=True):
                # one [128,512] y tile: 2 Wo matmuls + copy into a per-row
                # staging buffer; the whole [128,2048] row block ships as one
                # DMA (SP holds its SEQ for the full transfer; Pool's SWDGE
                # is async, so alternate).
                qts = slice(qt * 128, (qt + 1) * 128)
                ns = slice(nb * QB, (nb + 1) * QB)
                if nb == 0:
                    ystage[qt] = sbY.tile([128, D // QB, QB], BF16, tag="ysb",
                                          name=f"ystage{qt}")
                psum_y = psY.tile([128, QB], F32, tag="y")
                for h in range(2):
                    _lbl(nc.tensor.matmul(
                        psum_y[:],
                        oT[h][qt // 4][:, (qt % 4) * 128:(qt % 4 + 1) * 128],
                        wo_sb[:, h, ns],
                        start=(h == 0), stop=(h == 1)),
                        f"y_qt{qt}_nb{nb}_h{h}")
                # copy engine: Act where the current group's exp load is
                # light (act_ok), else mostly DVE
                if ycnt[0] % (2 if (act_share or act_ok) else 4) == 1:
                    nc.scalar.copy(ystage[qt][:, nb, :], psum_y[:])
                else:
                    nc.vector.tensor_copy(ystage[qt][:, nb, :], psum_y[:])
                ycnt[0] += 1
                # Pool SWDGE (async) for in-group rows; final-drain rows are
                # split across SP/Act/Pool queues piece-by-piece so the last
                # flush is a single small racing transfer per queue
                # ship pieces as soon as their copies land: SP first half,
                # SP third quarter, Pool final quarter — the last in-flight
                # piece is small so the end flush chain is short
                if nb == 1:
                    nc.sync.dma_start(y_d[qts, 0:D // 2], ystage[qt][:, 0:2, :])
                elif nb == 2:
                    nc.sync.dma_start(y_d[qts, D // 2:3 * D // 4],
                                      ystage[qt][:, 2, :])
                elif nb == 3:
                    nc.gpsimd.dma_start(y_d[qts, 3 * D // 4:D],
                                        ystage[qt][:, 3, :])
                    del ystage[qt]
            return emit_y_tile

        with tc.tile_pool(name="psS", bufs=4, space="PSUM") as psS, \
             tc.tile_pool(name="psO", bufs=2, space="PSUM") as psO, \
             tc.tile_pool(name="psD", bufs=1, space="PSUM") as psD, \
             tc.tile_pool(name="psY", bufs=1, space="PSUM") as psY, \
             tc.tile_pool(name="epool", bufs=8) as epool, \
             tc.tile_pool(name="sbB", bufs=4) as sbB, \
             tc.tile_pool(name="sbY", bufs=4) as sbY:

            emit_y_tile = make_emit_y(psY, sbY, act_share=False)

            pending_y = []
            LAG = 3  # tiles between score emission and its den/AV, hiding exp
            for qb in range(NQB):
                qs = slice(qb * QB, (qb + 1) * QB)
                for h in range(2):
                    acts = [(kt, _classify(kt, qb)) for kt in range(NKT)]
                    acts = [(kt, c, mi) for kt, (c, mi) in acts if c != "skip"]
                    # full tiles first: partial tiles' den/AV depend on DVE
                    # mask-muls, so give DVE the whole group to produce them
                    acts = ([a for a in acts if a[1] == "full"]
                            + [a for a in acts if a[1] == "partial"])
                    n_act = len(acts)
                    first_group = (qb == 0 and h == 0)
                    psum_o = psO.tile([128, QB], F32, tag="o")
                    psum_den = psD.tile([1, QB], F32, tag="den")
                    e_use = [None] * n_act
                    for i in range(n_act + LAG):
                        if i < n_act:
                            kt, cls, mi = acts[i]
                            psum_s = psS.tile([128, QB], F32, tag="s")
                            _lbl(nc.tensor.matmul(
                                psum_s[:],
                                kT[:, kt * 128:(kt + 1) * 128],
                                qT[h][:, qs],
                                start=True, stop=True),
                                f"score_h{h}_qb{qb}_kt{kt}")
                            e_sb = epool.tile([128, QB], BF16, tag="e")
                            nc.scalar.activation(
                                e_sb[:], psum_s[:],
                                mybir.ActivationFunctionType.Exp,
                                scale=SM_SCALE)
                            if cls == "partial":
                                e_m = epool.tile([128, QB], BF16, tag="em")
                                nc.vector.tensor_mul(
                                    e_m[:], e_sb[:], masks[:, mi, :])
                                e_use[i] = e_m
                            else:
                                e_use[i] = e_sb
                        if first_group and i == 4:
                            # deferred last-block v transposes, borrowing a
                            # score-pool buffer (same bank size)
                            v_transposes(NLB - 1, psS, tag="s")
                        # interleave one deferred y tile of the previous q
                        # block BEFORE den/AV: fills PE while the exp chain
                        # completes and issues the staging copy earlier
                        if pending_y and i >= 2 and not (qb == NQB - 1
                                                         and h == 1):
                            # late q-blocks run 12-tile groups: Act has slack.
                            # nothing pops in the very last group — leftovers
                            # drain through the deep-buffered pools instead
                            emit_y_tile(*pending_y.pop(0), act_ok=(qb >= 2))
                        j = i - LAG
                        if 0 <= j < n_act:
                            ktj = acts[j][0]
                            _lbl(nc.tensor.matmul(
                                psum_den[:], ones_bf[:],
                                e_use[j][:],
                                start=(j == 0), stop=(j == n_act - 1)),
                                f"den_h{h}_qb{qb}_kt{ktj}")
                            _lbl(nc.tensor.matmul(
                                psum_o[:], v_sb[:, ktj, :],
                                e_use[j][:],
                                start=(j == 0), stop=(j == n_act - 1)),
                                f"av_h{h}_qb{qb}_kt{ktj}")
                            e_use[j] = None
                    den_sb = sbB.tile([1, QB], F32, tag="densb")
                    nc.scalar.activation(
                        den_sb[:], psum_den[:],
                        mybir.ActivationFunctionType.Identity,
                        bias=exps_sb[0:1, h:h + 1])
                    r_sb = sbB.tile([1, QB], F32, tag="rsb")
                    nc.vector.reciprocal(r_sb[:], den_sb[:])
                    rb = sbB.tile([128, QB], F32, tag="rb")
                    nc.gpsimd.partition_broadcast(rb[:], r_sb[:])
                    nc.vector.tensor_mul(oT[h][qb][:], psum_o[:], rb[:])
                pending_y.extend(
                    (qb * (QB // 128) + j, nb)
                    for j in range(QB // 128) for nb in range(D // QB))

        # final-qb y drain: attention pools are done, so rebuild with deep
        # buffering and let the copies use both Act and DVE
        with tc.tile_pool(name="psY2", bufs=4, space="PSUM") as psY2, \
             tc.tile_pool(name="sbY2", bufs=4) as sbY2:
            emit_y_tile = make_emit_y(psY2, sbY2, act_share=True)
            while pending_y:
                emit_y_tile(*pending_y.pop(0))

        if dump:
            with tc.tile_pool(name="dbgp", bufs=2) as dbgp:
                for nm, t in (("dbg_q0", qT[0]), ("dbg_q1", qT[1]),
                              ("dbg_k", kT), ("dbg_vT", vT)):
                    f = dbgp.tile([128, L], F32, tag="dbgf", name=f"f{nm}")
                    nc.scalar.copy(f[:], t[:])
                    nc.sync.dma_start(dbg[nm], f[:])
                for h in range(2):
                    f = dbgp.tile([128, L], F32, tag="dbgf", name=f"fo{h}")
                    for b in range(NQB):
                        nc.scalar.copy(f[:, b * QB:(b + 1) * QB], oT[h][b][:])
                    nc.sync.dma_start(dbg[f"dbg_o{h}"], f[:])
                fv = dbgp.tile([128, NKT, HD], F32, tag="dbgf", name="fv")
                nc.scalar.copy(fv[:], v_sb[:])
                nc.sync.dma_start(dbg["dbg_v"], fv[:])

    nc.compile()
    return nc


def _rope_tables():
    """cos/sin tables in the permuted row order (see ROPE_PERM)."""
    freqs = (1.0 / ROPE_BASE) ** np.linspace(0.0, 1.0, num=HD // 4,
                                             dtype=np.float32)
    theta = freqs[:, None].astype(np.float32) * np.arange(L, dtype=np.float32)[None, :]
    cos32 = np.cos(theta).astype(np.float32)   # (32, L), freq j
    sin32 = np.sin(theta).astype(np.float32)
    cosd = np.ones((128, L), dtype=np.float32)
    sind = np.zeros((128, L), dtype=np.float32)
    for sq in range(2):   # quadrants 0,1 carry the 32 active freqs
        fr = slice(16 * sq, 16 * sq + 16)
        cosd[32 * sq:32 * sq + 16] = cos32[fr]
        cosd[32 * sq + 16:32 * sq + 32] = cos32[fr]
        sind[32 * sq:32 * sq + 16] = sin32[fr]
        sind[32 * sq + 16:32 * sq + 32] = -sin32[fr]
    return cosd, sind


def _make_in_maps(x, Wqkv, Wo, s):
    bf16 = mybir.dt.np(BF16)
    x = np.asarray(x, dtype=np.float32)
    Wqkv = np.asarray(Wqkv, dtype=np.float32)
    Wo = np.asarray(Wo, dtype=np.float32)
    s = np.asarray(s, dtype=np.float32)
    # pack to SBUF layout [partition, chunk, col]: xT[p, k, l] = x[l, k*128+p]
    xT = np.ascontiguousarray(
        x.reshape(L, NDK, 128).transpose(2, 1, 0)).astype(bf16)
    cosd, sind = _rope_tables()
    cosd = cosd.astype(bf16)
    sind = sind.astype(bf16)
    in_maps = []
    for c in range(N_CORES):
        g = c // 2
        wslc = np.concatenate([
            Wqkv[:, (2 * c) * HD:(2 * c) * HD + HD][:, ROPE_PERM],
            Wqkv[:, (2 * c + 1) * HD:(2 * c + 2) * HD][:, ROPE_PERM],
            Wqkv[:, 16 * HD + g * HD:16 * HD + (g + 1) * HD][:, ROPE_PERM],
            Wqkv[:, 20 * HD + g * HD:20 * HD + (g + 1) * HD],
        ], axis=1)
        wslc_p = np.ascontiguousarray(
            wslc.reshape(NDK, 128, 4 * HD).transpose(1, 0, 2)).astype(bf16)
        in_maps.append({
            "xT": xT,
            "wslc": wslc_p,
            "wo": np.ascontiguousarray(Wo[(2 * c) * HD:(2 * c + 2) * HD, :]).astype(bf16),
            "snk": np.ascontiguousarray(s[:, 2 * c:2 * c + 2]),
            "cosd": cosd,
            "sind": sind,
        })
    return in_maps


_CACHE = {}


def _get_exec():
    """Build the program once and return a cached jitted 8-core executor."""
    if "exec" in _CACHE:
        return _CACHE["exec"]

    import jax
    from jax.sharding import Mesh, PartitionSpec
    from jax.experimental.shard_map import shard_map
    from concourse.bass2jax import (_bass_exec_p, install_neuronx_cc_hook,
                                    partition_id_tensor)

    nc = _build_program()
    install_neuronx_cc_hook()

    partition_name = (nc.partition_id_tensor.name
                      if nc.partition_id_tensor else None)
    in_names, out_names, out_avals = [], [], []
    for alloc in nc.m.functions[0].allocations:
        if not isinstance(alloc, mybir.MemoryLocationSet):
            continue
        name = alloc.memorylocations[0].name
        if alloc.kind == "ExternalInput":
            if name != partition_name:
                in_names.append(name)
        elif alloc.kind == "ExternalOutput":
            out_names.append(name)
            out_avals.append(jax.core.ShapedArray(
                tuple(alloc.tensor_shape), mybir.dt.np(alloc.dtype)))
    n_params = len(in_names)
    all_names = in_names + out_names
    if partition_name is not None:
        all_names = all_names + [partition_name]

    def _body(*args):
        operands = list(args)
        if partition_name is not None:
            operands.append(partition_id_tensor())
        outs = _bass_exec_p.bind(
            *operands,
            out_avals=tuple(out_avals),
            in_names=tuple(all_names),
            out_names=tuple(out_names),
            lowering_input_output_aliases=(),
            sim_require_finite=True,
            sim_require_nnan=True,
            nc=nc,
        )
        return tuple(outs)

    devices = jax.devices()[:N_CORES]
    mesh = Mesh(np.asarray(devices), ("core",))
    n_outs = len(out_names)
    sharded = jax.jit(
        shard_map(_body, mesh=mesh,
                  in_specs=(PartitionSpec("core"),) * (n_params + n_outs),
                  out_specs=(PartitionSpec("core"),) * n_outs,
                  check_rep=False),
        keep_unused=True)

    state = {
        "sharded": sharded, "in_names": in_names, "out_names": out_names,
        "out_avals": out_avals, "mesh": mesh, "n_params": n_params,
        "nc": nc,
    }
    _CACHE["exec"] = state
    return state


def _run_cores(in_maps):
    ex = _get_exec()
    concat_in = [
        np.concatenate([np.asarray(m[name]) for m in in_maps], axis=0)
        for name in ex["in_names"]
    ]
    concat_zeros = [
        np.zeros((N_CORES * a.shape[0],) + tuple(a.shape[1:]), a.dtype)
        for a in ex["out_avals"]
    ]
    outs = ex["sharded"](*concat_in, *concat_zeros)
    name_to_i = {n: i for i, n in enumerate(ex["out_names"])}
    yi = name_to_i["y"]
    y_all = np.asarray(outs[yi]).astype(np.float32).reshape(N_CORES, L, D)
    return y_all


def kernel(x, Wqkv, Wo, s):
    in_maps = _make_in_maps(x, Wqkv, Wo, s)
    y_all = _run_cores(in_maps)
    out = y_all.sum(axis=0, dtype=np.float32)
    return out.reshape(1, L, D).astype(np.float32)


# revision 91
# speedup vs baseline: 1.5137x; 1.0515x over previous
"""Trainium2 Bass kernel for GroupedQueryAttention (inverted sliding-window mask + sink).

Full inputs in, full output out. Internally head-sharded across 8 NeuronCores:
core c handles q heads {2c, 2c+1} and kv head c//2, computes its partial
(x @ Wqkv_slice -> RoPE -> scores -> masked softmax w/ sink -> AV -> @ Wo_slice),
host sums the 8 partial outputs (the all-reduce).

v2: bf16 matmul inputs (host-cast), streamed x loads, post-exp multiplicative
masks on DVE, stream_shuffle RoPE, Wo tiles interleaved into next q-block.
"""

import os
import sys
from contextlib import ExitStack

sys.path.insert(0, "/opt/trn_rl_repo")

# jax must see the axon/neuron platform; a stray JAX_PLATFORMS=cpu would hide it.
if os.environ.get("JAX_PLATFORMS", "") == "cpu":
    os.environ["JAX_PLATFORMS"] = ""

import numpy as np

import concourse.bass as bass
import concourse.tile as tile
from concourse import bacc, mybir

F32 = mybir.dt.float32
BF16 = mybir.dt.bfloat16

LABELS = {}  # instruction name -> logical label (for trace analysis)


def _lbl(inst, label):
    try:
        LABELS[inst.ins.name] = label
    except Exception:
        pass
    return inst

N_CORES = 8
L = 2048
D = 2048
HD = 128
WINDOW = 1024
ROPE_BASE = 1024.0
SM_SCALE = 1.0 / float(np.sqrt(HD))

QB = 512          # q block (free dim of score tiles)
NQB = L // QB     # 4
NKT = L // HD     # 16 k tiles of 128
NDK = D // HD     # 16 contraction chunks for projections
NLB = L // QB     # 4 l-blocks for projection

# multiplicative-mask tiles are keyed by diff0 = q0 - k0 of the (k-tile, q-block)
MASK_DIFF0S = [0, -128, -256, -384, 640, 768, 896, 1024]
MASK_IDX = {d: i for i, d in enumerate(MASK_DIFF0S)}

# stream_shuffle permutes only within 32-partition quadrants, so q/k head
# dims are re-ordered (host side) to put each RoPE pair (d, d+64) 16 rows
# apart inside one quadrant: row 32s+i -> dim 16s+i, row 32s+16+i -> dim
# 64+16s+i (i<16). The shared permutation leaves q.k dot products unchanged.
SHUF16 = [(i + 16) % 32 for i in range(32)]
ROPE_PERM = np.array(
    [16 * s + i if i < 16 else 64 + 16 * s + (i - 16)
     for s in range(4) for i in range(32)])


def _classify(kt: int, qb: int):
    """masked band is 0 <= q-k <= WINDOW-1 (those entries get zeroed).

    Returns (class, mask_idx, active_cols, triangle_cols): edge tiles have
    whole dead column ranges (all 128 k rows masked), so scores/exp/den/AV
    only touch `active_cols`; the mask-multiply only needs `triangle_cols`.
    """
    d0 = QB * qb - HD * kt
    if 128 <= d0 <= 512:
        return "skip", None, None, None
    if d0 <= -512 or d0 >= 1152:
        return "full", None, (0, QB), None
    mi = MASK_IDX[d0]
    if d0 <= 0:
        # active iff q-k < 0, i.e. qf < kp - d0: cols [0, -d0+128)
        c = -d0
        return "partial", mi, (0, min(c + 128, QB)), (c, min(c + 128, QB))
    # active iff q-k >= WINDOW, i.e. qf >= kp + (WINDOW - d0)
    c = WINDOW - d0
    return "partial", mi, (c, QB), (c, min(c + 128, QB))


def _build_program(dump=False):
    nc = bacc.Bacc("TRN2", target_bir_lowering=False, debug=False,
                   num_devices=N_CORES)
    dbg = {}
    if dump:
        for nm in ("dbg_q0", "dbg_q1", "dbg_k", "dbg_vT", "dbg_o0", "dbg_o1"):
            dbg[nm] = nc.dram_tensor(nm, [128, L], F32, kind="ExternalOutput").ap()
        dbg["dbg_v"] = nc.dram_tensor("dbg_v", [128, NKT * HD], F32,
                                      kind="ExternalOutput").ap()

    # host-packed to SBUF layout: [partition, chunk, col]
    xT_d = nc.dram_tensor("xT", [128, NDK, L], BF16, kind="ExternalInput").ap()
    wslc_d = nc.dram_tensor("wslc", [128, NDK, 4 * HD], BF16,
                            kind="ExternalInput").ap()
    wo_d = nc.dram_tensor("wo", [2 * HD, D], BF16, kind="ExternalInput").ap()
    snk_d = nc.dram_tensor("snk", [1, 2], F32, kind="ExternalInput").ap()
    cosd_d = nc.dram_tensor("cosd", [128, L], BF16, kind="ExternalInput").ap()
    sind_d = nc.dram_tensor("sind", [128, L], BF16, kind="ExternalInput").ap()
    # partial y ships bf16 (halves the dominant write traffic); host sums f32
    y_d = nc.dram_tensor("y", [L, D], BF16, kind="ExternalOutput").ap()

    with tile.TileContext(nc) as tc, ExitStack() as stk:
        persist = stk.enter_context(tc.tile_pool(name="persist", bufs=1))

        # ---- persistent SBUF tensors ----
        wslc_sb = persist.tile([128, NDK, 4 * HD], BF16, tag="wslc")
        wo_sb = persist.tile([128, 2, D], BF16, tag="wo")
        qT = [persist.tile([128, L], BF16, tag=f"qT{h}", name=f"qT{h}") for h in range(2)]
        kT = persist.tile([128, L], BF16, tag="kT")
        vT = persist.tile([128, L], BF16, tag="vT")
        v_sb = persist.tile([128, NKT, HD], BF16, tag="v")
        # one tile per (head, q-block): slices of a single tensor would
        # false-couple interleaved Wo reads to later rb-mul writes
        oT = [[persist.tile([128, QB], BF16, tag=f"oT{h}_{b}", name=f"oT{h}_{b}")
               for b in range(NQB)] for h in range(2)]
        cosd_sb = persist.tile([128, L], BF16, tag="cosd")
        sind_sb = persist.tile([128, L], BF16, tag="sind")
        masks = persist.tile([128, len(MASK_DIFF0S), QB], BF16, tag="masks")
        ident = persist.tile([128, 128], BF16, tag="ident")
        ones_bf = persist.tile([128, 1], BF16, tag="ones")
        snk_sb = persist.tile([1, 2], F32, tag="snk")
        exps_sb = persist.tile([1, 2], F32, tag="exps")

        # ---- weight loads ----
        # wslc split across SP and Act queues in small pieces so the
        # transfers interleave with the x-chunk stream on the DMA engines
        # (per-DMA queue hold is ~1.3us + transfer, so one queue can't feed
        # a chunk every 0.85us alone)
        for eng, pieces in ((nc.sync, ((0, 1), (2, 4), (6, 8), (8, 10),
                                       (10, 12), (12, 14), (14, 16))),
                            (nc.scalar, ((1, 2), (4, 6)))):
            for a, b in pieces:
                eng.dma_start(wslc_sb[:, a:b, :], wslc_d[:, a:b, :])
        nc.scalar.dma_start(snk_sb[:], snk_d[:])

        nc.vector.memset(ones_bf[:], 1.0)
        # exp of the two sink logits
        nc.scalar.activation(exps_sb[:], snk_sb[:], mybir.ActivationFunctionType.Exp)

        def build_masks():
            # multiplicative masks: 0 where 0 <= (q-k) <= WINDOW-1, else 1
            # (on Pool, emitted mid-phase-A: needed only at phase B)
            for i, d0 in enumerate(MASK_DIFF0S):
                m = masks[:, i, :]
                nc.gpsimd.memset(m, 1.0)
                if d0 <= 0:
                    # keep 1 where q-k < 0, i.e. kp - qf - d0 - 1 >= 0
                    nc.gpsimd.affine_select(
                        out=m, in_=m, compare_op=mybir.AluOpType.is_ge,
                        fill=0.0, base=-d0 - 1, channel_multiplier=1,
                        pattern=[[-1, QB]])
                else:
                    # keep 1 where q-k >= WINDOW, i.e. qf - kp + d0 - WINDOW >= 0
                    nc.gpsimd.affine_select(
                        out=m, in_=m, compare_op=mybir.AluOpType.is_ge,
                        fill=0.0, base=d0 - WINDOW, channel_multiplier=-1,
                        pattern=[[1, QB]])

        # ================= Phase A: QKV projection (transposed) =================
        # pT[c*128+r, l] = sum_d wslc[d, c*128+r] * x[l, d];  cols c = q0,q1,k,v
        col_dst = [qT[0], qT[1], kT, vT]

        xt_sb = persist.tile([128, NDK, L], BF16, tag="xt")

        # rope pool spans phases A and B: the last l-block's RoPE and
        # transposes are deferred into early attention
        rp = stk.enter_context(tc.tile_pool(name="rope", bufs=4))

        def rope_slice(t, lb):
            ls = slice(lb * QB, (lb + 1) * QB)
            partner = rp.tile([128, QB], BF16, tag="partner")
            nc.vector.stream_shuffle(partner[:], t[:, ls], SHUF16)
            tmp = rp.tile([128, QB], BF16, tag="ropetmp")
            nc.vector.tensor_mul(tmp[:], t[:, ls], cosd_sb[:, ls])
            nc.vector.tensor_mul(partner[:], partner[:], sind_sb[:, ls])
            nc.vector.tensor_add(t[:, ls], tmp[:], partner[:])

        def v_transposes(lb, pool, tag="vt"):
            pt = pool.tile([128, 4, HD], BF16, tag=tag)
            for j in range(4):
                t = 4 * lb + j
                _lbl(nc.tensor.transpose(
                    pt[:, j, :], vT[:, t * 128:(t + 1) * 128], ident[:]),
                    f"transp{t}")
            # DVE, not Act: Act must stay clear to start the exps promptly
            # (Pool can't read PSUM)
            nc.vector.tensor_copy(v_sb[:, 4 * lb:4 * lb + 4, :], pt[:])

        with tc.tile_pool(name="psA", bufs=6, space="PSUM") as psA, \
             tc.tile_pool(name="psT", bufs=1, space="PSUM") as psT:

            prev_v_lb = None
            for lb in range(NLB):
                ls = slice(lb * QB, (lb + 1) * QB)
                # x loads ride Pool's software DGE: the transfer is async, so
                # the Pool engine is only held ~1us per DMA. First block goes
                # per-chunk so PE starts as soon as possible.
                if lb == 0:
                    # Pool SWDGE issues ~1us apart vs PE consuming a chunk
                    # every ~0.85us: single chunks first, then pairs
                    for a, b in ((0, 2), (2, 4), (4, 6), (6, 8),
                                 (8, 10), (10, 13), (13, 16)):
                        nc.gpsimd.dma_start(
                            xt_sb[:, a:b, ls], xT_d[:, a:b, ls])
                else:
                    for g in range(4):
                        nc.gpsimd.dma_start(
                            xt_sb[:, 4 * g:4 * g + 4, ls],
                            xT_d[:, 4 * g:4 * g + 4, ls])
                # cos/sin slices arrive just before this block's RoPE; wo is
                # not needed until phase C — keep them off the early window
                nc.scalar.dma_start(cosd_sb[:, ls], cosd_d[:, ls])
                nc.scalar.dma_start(sind_sb[:, ls], sind_d[:, ls])
                if lb == 1:
                    for h in range(2):
                        nc.scalar.dma_start(wo_sb[:, h, :],
                                            wo_d[h * 128:(h + 1) * 128, :])
                if lb == 0:
                    # identity for PE transposes (needed from lb1 on)
                    nc.gpsimd.memset(ident[:], 0.0)
                    nc.gpsimd.affine_select(
                        out=ident[:], in_=ident[:],
                        compare_op=mybir.AluOpType.not_equal,
                        fill=1.0, base=0, channel_multiplier=1,
                        pattern=[[-1, 128]])
                if lb == NLB - 1:
                    build_masks()
                if lb < NLB - 1:
                    # chunk-major: all 4 column psums fill together while the
                    # x chunks stream in
                    psums = [psA.tile([128, QB], F32, tag="proj",
                                      name=f"psproj{c}") for c in range(4)]
                    for k in range(NDK):
                        for c in range(4):
                            _lbl(nc.tensor.matmul(
                                psums[c][:],
                                wslc_sb[:, k, c * 128:(c + 1) * 128],
                                xt_sb[:, k, ls],
                                start=(k == 0), stop=(k == NDK - 1)),
                                f"proj_lb{lb}_k{k}_c{c}")
                    if prev_v_lb is not None:
                        v_transposes(prev_v_lb, psT)
                    # copies psum -> bf16 SBUF; k,q0 on Act; q1,v on DVE
                    nc.scalar.copy(kT[:, ls], psums[2][:])
                    nc.scalar.copy(qT[0][:, ls], psums[0][:])
                    nc.vector.tensor_copy(qT[1][:, ls], psums[1][:])
                    nc.vector.tensor_copy(vT[:, ls], psums[3][:])
                    rope_slice(kT, lb)
                    rope_slice(qT[0], lb)
                    rope_slice(qT[1], lb)
                else:
                    # last block goes column-major (its x data is resident
                    # already): each column's copy + RoPE overlaps the next
                    # column's matmuls, so the attention pool-open barrier
                    # only waits on the final v copy
                    for c, dst, eng, rope in ((2, kT, nc.scalar, True),
                                              (0, qT[0], nc.scalar, True),
                                              (1, qT[1], nc.vector, True),
                                              (3, vT, nc.vector, False)):
                        ps = psA.tile([128, QB], F32, tag="proj",
                                      name=f"pscol{c}")
                        for k in range(NDK):
                            _lbl(nc.tensor.matmul(
                                ps[:],
                                wslc_sb[:, k, c * 128:(c + 1) * 128],
                                xt_sb[:, k, ls],
                                start=(k == 0), stop=(k == NDK - 1)),
                                f"proj_lb{lb}_k{k}_c{c}")
                        if c == 2 and prev_v_lb is not None:
                            v_transposes(prev_v_lb, psT)
                        if eng is nc.scalar:
                            nc.scalar.copy(dst[:, ls], ps[:])
                        else:
                            nc.vector.tensor_copy(dst[:, ls], ps[:])
                        if rope:
                            rope_slice(dst, lb)
                prev_v_lb = lb

        # ============ Phase B+C: attention + output projection ============
        ycnt = [0]
        ystage = {}

        def make_emit_y(psY, sbY, act_share):
            def emit_y_tile(qt, nb, act_ok=True):
                # one [128,512] y tile: 2 Wo matmuls + copy into a per-row
                # staging buffer; the whole [128,2048] row block ships as one
                # DMA (SP holds its SEQ for the full transfer; Pool's SWDGE
                # is async, so alternate).
                qts = slice(qt * 128, (qt + 1) * 128)
                ns = slice(nb * QB, (nb + 1) * QB)
                if nb == 0:
                    ystage[qt] = sbY.tile([128, D // QB, QB], BF16, tag="ysb",
                                          name=f"ystage{qt}")
                psum_y = psY.tile([128, QB], F32, tag="y")
                for h in range(2):
                    _lbl(nc.tensor.matmul(
                        psum_y[:],
                        oT[h][qt // 4][:, (qt % 4) * 128:(qt % 4 + 1) * 128],
                        wo_sb[:, h, ns],
                        start=(h == 0), stop=(h == 1)),
                        f"y_qt{qt}_nb{nb}_h{h}")
                # copy engine: Act where the current group's exp load is
                # light (act_ok), else mostly DVE
                if ycnt[0] % (2 if (act_share or act_ok) else 4) == 1:
                    nc.scalar.copy(ystage[qt][:, nb, :], psum_y[:])
                else:
                    nc.vector.tensor_copy(ystage[qt][:, nb, :], psum_y[:])
                ycnt[0] += 1
                # Pool SWDGE (async) for in-group rows; final-drain rows are
                # split across SP/Act/Pool queues piece-by-piece so the last
                # flush is a single small racing transfer per queue
                # ship pieces as soon as their copies land: SP first half,
                # SP third quarter, Pool final quarter — the last in-flight
                # piece is small so the end flush chain is short
                if nb == 1:
                    nc.sync.dma_start(y_d[qts, 0:D // 2], ystage[qt][:, 0:2, :])
                elif nb == 2:
                    nc.sync.dma_start(y_d[qts, D // 2:3 * D // 4],
                                      ystage[qt][:, 2, :])
                elif nb == 3:
                    nc.gpsimd.dma_start(y_d[qts, 3 * D // 4:D],
                                        ystage[qt][:, 3, :])
                    del ystage[qt]
            return emit_y_tile

        with tc.tile_pool(name="psS", bufs=4, space="PSUM") as psS, \
             tc.tile_pool(name="psO", bufs=2, space="PSUM") as psO, \
             tc.tile_pool(name="psD", bufs=1, space="PSUM") as psD, \
             tc.tile_pool(name="psY", bufs=1, space="PSUM") as psY, \
             tc.tile_pool(name="epool", bufs=8) as epool, \
             tc.tile_pool(name="sbB", bufs=4) as sbB, \
             tc.tile_pool(name="sbY", bufs=4) as sbY:

            emit_y_tile = make_emit_y(psY, sbY, act_share=False)

            pending_y = []
            LAG = 3  # tiles between score emission and its den/AV, hiding exp
            for qb in range(NQB):
                qs = slice(qb * QB, (qb + 1) * QB)
                for h in range(2):
                    acts = [(kt,) + _classify(kt, qb) for kt in range(NKT)]
                    acts = [a for a in acts if a[1] != "skip"]
                    # full tiles first: partial tiles' den/AV depend on DVE
                    # mask-muls, so give DVE the whole group to produce them.
                    # The first (full) tile also initializes the whole den/o
                    # psum regions that later sliced tiles accumulate into.
                    acts = ([a for a in acts if a[1] == "full"]
                            + [a for a in acts if a[1] == "partial"])
                    n_act = len(acts)
                    first_group = (qb == 0 and h == 0)
                    psum_o = psO.tile([128, QB], F32, tag="o")
                    psum_den = psD.tile([1, QB], F32, tag="den")
                    e_use = [None] * n_act
                    for i in range(n_act + LAG):
                        if i < n_act:
                            kt, cls, mi, (a0, a1), tri = acts[i]
                            asl = slice(qb * QB + a0, qb * QB + a1)
                            psum_s = psS.tile([128, QB], F32, tag="s")
                            _lbl(nc.tensor.matmul(
                                psum_s[:, a0:a1],
                                kT[:, kt * 128:(kt + 1) * 128],
                                qT[h][:, asl],
                                start=True, stop=True),
                                f"score_h{h}_qb{qb}_kt{kt}")
                            e_sb = epool.tile([128, QB], BF16, tag="e")
                            nc.scalar.activation(
                                e_sb[:, a0:a1], psum_s[:, a0:a1],
                                mybir.ActivationFunctionType.Exp,
                                scale=SM_SCALE)
                            if cls == "partial":
                                t0, t1 = tri
                                nc.vector.tensor_mul(
                                    e_sb[:, t0:t1], e_sb[:, t0:t1],
                                    masks[:, mi, t0:t1])
                            e_use[i] = e_sb
                        if first_group and i == 4:
                            # deferred last-block v transposes, borrowing a
                            # score-pool buffer (same bank size)
                            v_transposes(NLB - 1, psS, tag="s")
                        # interleave one deferred y tile of the previous q
                        # block BEFORE den/AV: fills PE while the exp chain
                        # completes and issues the staging copy earlier
                        pop_ok = (i >= 2 and not (qb == NQB - 1 and h == 1)
                                  and (qb < NQB - 1 or i % 2 == 0))
                        if pending_y and pop_ok:
                            # late q-blocks run short sliced groups: space the
                            # pops out there (the single Wo psum bank stalls
                            # on back-to-back pops) and keep the very last
                            # group clear — leftovers drain through the
                            # deep-buffered pools instead
                            emit_y_tile(*pending_y.pop(0), act_ok=(qb >= 2))
                        j = i - LAG
                        if 0 <= j < n_act:
                            ktj = acts[j][0]
                            b0, b1 = acts[j][3]
                            _lbl(nc.tensor.matmul(
                                psum_den[0:1, b0:b1], ones_bf[:],
                                e_use[j][:, b0:b1],
                                start=(j == 0), stop=(j == n_act - 1)),
                                f"den_h{h}_qb{qb}_kt{ktj}")
                            _lbl(nc.tensor.matmul(
                                psum_o[:, b0:b1], v_sb[:, ktj, :],
                                e_use[j][:, b0:b1],
                                start=(j == 0), stop=(j == n_act - 1)),
                                f"av_h{h}_qb{qb}_kt{ktj}")
                            e_use[j] = None
                    den_sb = sbB.tile([1, QB], F32, tag="densb")
                    nc.scalar.activation(
                        den_sb[:], psum_den[:],
                        mybir.ActivationFunctionType.Identity,
                        bias=exps_sb[0:1, h:h + 1])
                    r_sb = sbB.tile([1, QB], F32, tag="rsb")
                    nc.vector.reciprocal(r_sb[:], den_sb[:])
                    rb = sbB.tile([128, QB], F32, tag="rb")
                    nc.gpsimd.partition_broadcast(rb[:], r_sb[:])
                    nc.vector.tensor_mul(oT[h][qb][:], psum_o[:], rb[:])
                pending_y.extend(
                    (qb * (QB // 128) + j, nb)
                    for j in range(QB // 128) for nb in range(D // QB))

        # final-qb y drain: attention pools are done, so rebuild with deep
        # buffering and let the copies use both Act and DVE
        with tc.tile_pool(name="psY2", bufs=4, space="PSUM") as psY2, \
             tc.tile_pool(name="sbY2", bufs=4) as sbY2:
            emit_y_tile = make_emit_y(psY2, sbY2, act_share=True)
            while pending_y:
                emit_y_tile(*pending_y.pop(0))

        if dump:
            with tc.tile_pool(name="dbgp", bufs=2) as dbgp:
                for nm, t in (("dbg_q0", qT[0]), ("dbg_q1", qT[1]),
                              ("dbg_k", kT), ("dbg_vT", vT)):
                    f = dbgp.tile([128, L], F32, tag="dbgf", name=f"f{nm}")
                    nc.scalar.copy(f[:], t[:])
                    nc.sync.dma_start(dbg[nm], f[:])
                for h in range(2):
                    f = dbgp.tile([128, L], F32, tag="dbgf", name=f"fo{h}")
                    for b in range(NQB):
                        nc.scalar.copy(f[:, b * QB:(b + 1) * QB], oT[h][b][:])
                    nc.sync.dma_start(dbg[f"dbg_o{h}"], f[:])
                fv = dbgp.tile([128, NKT, HD], F32, tag="dbgf", name="fv")
                nc.scalar.copy(fv[:], v_sb[:])
                nc.sync.dma_start(dbg["dbg_v"], fv[:])

    nc.compile()
    return nc


def _rope_tables():
    """cos/sin tables in the permuted row order (see ROPE_PERM)."""
    freqs = (1.0 / ROPE_BASE) ** np.linspace(0.0, 1.0, num=HD // 4,
                                             dtype=np.float32)
    theta = freqs[:, None].astype(np.float32) * np.arange(L, dtype=np.float32)[None, :]
    cos32 = np.cos(theta).astype(np.float32)   # (32, L), freq j
    sin32 = np.sin(theta).astype(np.float32)
    cosd = np.ones((128, L), dtype=np.float32)
    sind = np.zeros((128, L), dtype=np.float32)
    for sq in range(2):   # quadrants 0,1 carry the 32 active freqs
        fr = slice(16 * sq, 16 * sq + 16)
        cosd[32 * sq:32 * sq + 16] = cos32[fr]
        cosd[32 * sq + 16:32 * sq + 32] = cos32[fr]
        sind[32 * sq:32 * sq + 16] = sin32[fr]
        sind[32 * sq + 16:32 * sq + 32] = -sin32[fr]
    return cosd, sind


def _make_in_maps(x, Wqkv, Wo, s):
    bf16 = mybir.dt.np(BF16)
    x = np.asarray(x, dtype=np.float32)
    Wqkv = np.asarray(Wqkv, dtype=np.float32)
    Wo = np.asarray(Wo, dtype=np.float32)
    s = np.asarray(s, dtype=np.float32)
    # pack to SBUF layout [partition, chunk, col]: xT[p, k, l] = x[l, k*128+p]
    xT = np.ascontiguousarray(
        x.reshape(L, NDK, 128).transpose(2, 1, 0)).astype(bf16)
    cosd, sind = _rope_tables()
    cosd = cosd.astype(bf16)
    sind = sind.astype(bf16)
    in_maps = []
    for c in range(N_CORES):
        g = c // 2
        wslc = np.concatenate([
            Wqkv[:, (2 * c) * HD:(2 * c) * HD + HD][:, ROPE_PERM],
            Wqkv[:, (2 * c + 1) * HD:(2 * c + 2) * HD][:, ROPE_PERM],
            Wqkv[:, 16 * HD + g * HD:16 * HD + (g + 1) * HD][:, ROPE_PERM],
            Wqkv[:, 20 * HD + g * HD:20 * HD + (g + 1) * HD],
        ], axis=1)
        wslc_p = np.ascontiguousarray(
            wslc.reshape(NDK, 128, 4 * HD).transpose(1, 0, 2)).astype(bf16)
        in_maps.append({
            "xT": xT,
            "wslc": wslc_p,
            "wo": np.ascontiguousarray(Wo[(2 * c) * HD:(2 * c + 2) * HD, :]).astype(bf16),
            "snk": np.ascontiguousarray(s[:, 2 * c:2 * c + 2]),
            "cosd": cosd,
            "sind": sind,
        })
    return in_maps


_CACHE = {}


def _get_exec():
    """Build the program once and return a cached jitted 8-core executor."""
    if "exec" in _CACHE:
        return _CACHE["exec"]

    import jax
    from jax.sharding import Mesh, PartitionSpec
    from jax.experimental.shard_map import shard_map
    from concourse.bass2jax import (_bass_exec_p, install_neuronx_cc_hook,
                                    partition_id_tensor)

    nc = _build_program()
    install_neuronx_cc_hook()

    partition_name = (nc.partition_id_tensor.name
                      if nc.partition_id_tensor else None)
    in_names, out_names, out_avals = [], [], []
    for alloc in nc.m.functions[0].allocations:
        if not isinstance(alloc, mybir.MemoryLocationSet):
            continue
        name = alloc.memorylocations[0].name
        if alloc.kind == "ExternalInput":
            if name != partition_name:
                in_names.append(name)
        elif alloc.kind == "ExternalOutput":
            out_names.append(name)
            out_avals.append(jax.core.ShapedArray(
                tuple(alloc.tensor_shape), mybir.dt.np(alloc.dtype)))
    n_params = len(in_names)
    all_names = in_names + out_names
    if partition_name is not None:
        all_names = all_names + [partition_name]

    def _body(*args):
        operands = list(args)
        if partition_name is not None:
            operands.append(partition_id_tensor())
        outs = _bass_exec_p.bind(
            *operands,
            out_avals=tuple(out_avals),
            in_names=tuple(all_names),
            out_names=tuple(out_names),
            lowering_input_output_aliases=(),
            sim_require_finite=True,
            sim_require_nnan=True,
            nc=nc,
        )
        return tuple(outs)

    devices = jax.devices()[:N_CORES]
    mesh = Mesh(np.asarray(devices), ("core",))
    n_outs = len(out_names)
    sharded = jax.jit(
        shard_map(_body, mesh=mesh,
                  in_specs=(PartitionSpec("core"),) * (n_params + n_outs),
                  out_specs=(PartitionSpec("core"),) * n_outs,
                  check_rep=False),
        keep_unused=True)

    state = {
        "sharded": sharded, "in_names": in_names, "out_names": out_names,
        "out_avals": out_avals, "mesh": mesh, "n_params": n_params,
        "nc": nc,
    }
    _CACHE["exec"] = state
    return state


def _run_cores(in_maps):
    ex = _get_exec()
    concat_in = [
        np.concatenate([np.asarray(m[name]) for m in in_maps], axis=0)
        for name in ex["in_names"]
    ]
    concat_zeros = [
        np.zeros((N_CORES * a.shape[0],) + tuple(a.shape[1:]), a.dtype)
        for a in ex["out_avals"]
    ]
    outs = ex["sharded"](*concat_in, *concat_zeros)
    name_to_i = {n: i for i, n in enumerate(ex["out_names"])}
    yi = name_to_i["y"]
    y_all = np.asarray(outs[yi]).astype(np.float32).reshape(N_CORES, L, D)
    return y_all


def kernel(x, Wqkv, Wo, s):
    in_maps = _make_in_maps(x, Wqkv, Wo, s)
    y_all = _run_cores(in_maps)
    out = y_all.sum(axis=0, dtype=np.float32)
    return out.reshape(1, L, D).astype(np.float32)
